# revision 28
# baseline (speedup 1.0000x reference)
"""Trainium2 Bass kernel: single-head attention + FFN transformer block.

Matmuls in bf16 except U (fp8e4 DoubleRow); S^T layout (math per batch b):
  S^T[k,q] = sum_d K[d,k] Q'[d,q] + kb[k]*qm[q]   (Q' = Q^T, invalid-q cols
                                                   zeroed host-side; kb = 0
                                                   valid / -1e30 masked)
  E[k,q]   = exp(S^T/32 - ln16)     ACT, fp8e4 out. Valid rows: softmax numer
                                    scaled by 1/16 (cancels in the ratio);
                                    invalid q: E=1/16 uniform over ALL k ->
                                    att = mean(V), matching the reference.
  rowsum   = ones^T E (PE),  recip via DVE, scattered to [q-part] layout.
  U[q,d]   = E^T_tile V      fp8 DoubleRow, K=256/MM. V is pair-interleaved
                             host-side (k-pair adjacent bytes) so the moving
                             operand streams N columns, not 2N.
  qres     = att + Q         fused on DVE (scalar_tensor_tensor), bf16
  y        = LN1(qres)       per-qt chain: bn_stats/aggr on DVE, then
                             rsqrt = exp(-0.5*ln(var+eps)) on ACT (the
                             natural_log_exp table holds both; the table-set
                             picker is pinned below so the whole kernel does
                             ONE ACT_TABLE_LOAD), nm = -mu*r as one DVE STT
  yT       = XBAR dma transpose per qt, ALL on the sync queue (concurrent
             transposes on two HWDGE queues race on the crossbar and corrupt
             blocks; the crossbar is also slow ~85GB/s, so only y uses it)
  H^T[o,q] = relu(W1T yT + b1)   bf16, W1 streamed as 8 JIT column chunks
  Z[q,d]   = H^T W2T;   out = LN2(y + Z)  (b2 cancels inside LN2)

Sharding: data-parallel, 4 batches per core on 8 cores.

Emission order (= scheduler priority): attn(b+1) is hoisted ahead of
ffn1(b) for b>=1 so its S/U matmuls fill the PE while ffn1(b) waits on the
y(b)->yT chain; attn(1) stays after ffn1(0) because batch 1's inputs are
still on the wire at that point. The last batch runs FFN2's LN2 per-qt to
shorten the drain tail.

Queues (engine FIFO head-of-line blocking is the enemy):
  sync   = all XBAR transposes + output stores + QT/KT bulk + W chunks
           (no compute engine -> nothing latency-critical stalls)
  scalar = ACT compute ONLY, zero DMA dispatches (a paced dispatch on a
           backed-up ring blocks every ACT op behind it in the FIFO)
  gpsimd = tiny mask loads + QTB/V8/Qf bulk (SWDGE, ~105GB/s)

PSUM: 4 S banks (reused by FFN2's Zps) + 2 U + 2 FFN1 = 8.
"""

import sys

sys.path.insert(0, "/opt/trn_rl_repo")

import numpy as np
import ml_dtypes

import concourse.bass as bass
import concourse.bacc as bacc
import concourse.mybir as mybir
from concourse import tile
from concourse.tile import add_dep_helper
from concourse.bass_utils import run_bass_kernel_spmd

# Force every activation this kernel uses (Exp, Ln, Relu, Identity) to
# resolve to the one table set that contains them all; the default set
# picker bounces between exp-only and ln-only sets, costing a ~1.3us
# ACT_TABLE_LOAD per LayerNorm rsqrt (39 loads = 50us of ACT time).
import concourse.hw_specs as _hw_specs

_ORIG_GAT = _hw_specs.get_activation_tables
_SHARED_SET = "natural_log_exp_and_others"


def _gat_pinned(module_arch):
    tabs = _ORIG_GAT(module_arch)
    af = mybir.ActivationFunctionType
    ours = {af.Exp, af.Ln, af.Relu, af.Identity, af.Copy}
    return {
        name: (funcs if name == _SHARED_SET else funcs - ours)
        for name, funcs in tabs.items()
    }


_hw_specs.get_activation_tables = _gat_pinned
bacc.get_activation_tables = _gat_pinned

B, QTL, KTL, D = 32, 512, 512, 1024
NCORES = 8
BL = B // NCORES
P = 128
NQT = QTL // P   # 4
NKT = KTL // P   # 4
NDT = D // P     # 8
NCH = 2          # 512-wide psum chunks per 1024
EPS = 1e-5
NEG = -1.0e30
LN16 = 2.772588722239781

F32 = mybir.dt.float32
F16 = mybir.dt.float16
BF16 = mybir.dt.bfloat16
F8 = mybir.dt.float8e4
AF = mybir.ActivationFunctionType
ALU = mybir.AluOpType
DR = mybir.MatmulPerfMode.DoubleRow
NP_BF16 = ml_dtypes.bfloat16
NP_F8 = ml_dtypes.float8_e4m3


def _build(apply1: bool, apply2: bool, b1f: float):
    nc = bacc.Bacc(None, target_bir_lowering=False)

    HQK = (NDT // 2) * KTL
    QTAd = nc.dram_tensor("QTAp", [BL, P, HQK], BF16, kind="ExternalInput")
    QTBd = nc.dram_tensor("QTBp", [BL, P, HQK], BF16, kind="ExternalInput")
    KTAd = nc.dram_tensor("KTAp", [BL, P, HQK], BF16, kind="ExternalInput")
    KTBd = nc.dram_tensor("KTBp", [BL, P, HQK], BF16, kind="ExternalInput")
    Vd = nc.dram_tensor("V8p", [BL, P, NKT * D], F8, kind="ExternalInput")
    Qd = nc.dram_tensor("Qp", [BL, P, NQT * D], BF16, kind="ExternalInput")
    QMd = nc.dram_tensor("QMp", [BL, QTL], BF16, kind="ExternalInput")
    KBd = nc.dram_tensor("KBp", [BL, KTL], BF16, kind="ExternalInput")
    W1Cd = nc.dram_tensor("W1Cp", [NDT, P, NDT * P], BF16,
                          kind="ExternalInput")
    W2Cd = nc.dram_tensor("W2Cp", [NCH, P, NDT * 512], BF16,
                          kind="ExternalInput")
    if apply1:
        G1d = nc.dram_tensor("G1p", [D], F32, kind="ExternalInput")
        B1d = nc.dram_tensor("B1p", [D], F32, kind="ExternalInput")
    if apply2:
        G2d = nc.dram_tensor("G2p", [D], F32, kind="ExternalInput")
        B2d = nc.dram_tensor("B2p", [D], F32, kind="ExternalInput")
    OUTd = nc.dram_tensor("OUTp", [BL, NQT, P, D], F16, kind="ExternalOutput")

    with tile.TileContext(nc) as tc:
        with (
            tc.tile_pool(name="const", bufs=1) as pc,
            tc.tile_pool(name="wts", bufs=1) as pw,
            tc.tile_pool(name="qkin", bufs=2) as pin,
            tc.tile_pool(name="mid", bufs=2) as pmid,
            tc.tile_pool(name="eh", bufs=1) as peh,
            tc.tile_pool(name="stream", bufs=2) as pst,
            tc.tile_pool(name="small", bufs=2) as psm,
            tc.tile_pool(name="psS", bufs=1, space="PSUM") as psS,
            tc.tile_pool(name="psU", bufs=2, space="PSUM") as psU,
            tc.tile_pool(name="psF", bufs=2, space="PSUM") as psF,
        ):
            # ---- constants / warmup ----
            wz = pc.tile([P, P], BF16)
            nc.vector.memset(wz, 0.0)
            wz512 = pc.tile([P, 512], BF16)
            nc.vector.memset(wz512, 0.0)
            onesb = pc.tile([P, 1], F8)
            nc.vector.memset(onesb, 1.0)
            nln16b = pc.tile([P, 1], F32)
            nc.vector.memset(nln16b, -LN16)
            s32b = pc.tile([P, 1], F32)
            nc.vector.memset(s32b, 1.0 / 32.0)
            b1b = pc.tile([P, 1], F32)
            nc.vector.memset(b1b, b1f)
            epsb = pc.tile([P, 1], F32)
            nc.vector.memset(epsb, EPS)
            nhalfb = pc.tile([P, 1], F32)
            nc.vector.memset(nhalfb, -0.5)
            if apply1:
                g1t = pc.tile([P, D], F32)
                nc.gpsimd.dma_start(
                    g1t, bass.AP(tensor=G1d, offset=0, ap=[[0, P], [1, D]]))
                b1t = pc.tile([P, D], F32)
                nc.gpsimd.dma_start(
                    b1t, bass.AP(tensor=B1d, offset=0, ap=[[0, P], [1, D]]))
            if apply2:
                g2t = pc.tile([P, D], F32)
                nc.gpsimd.dma_start(
                    g2t, bass.AP(tensor=G2d, offset=0, ap=[[0, P], [1, D]]))
                b2t = pc.tile([P, D], F32)
                nc.gpsimd.dma_start(
                    b2t, bass.AP(tensor=B2d, offset=0, ap=[[0, P], [1, D]]))

            W1Ct = [pw.tile([P, NDT, P], BF16, name=f"w1c{ot}")
                    for ot in range(NDT)]
            W2Ct = [pw.tile([P, NDT, 512], BF16, name=f"w2c{ch}")
                    for ch in range(NCH)]

            def load_batch(b):
                # tiny mask loads first (they gate the S-bias matmuls),
                # then bulk; sync stays free for transposes + stores.
                qmt = psm.tile([1, QTL], BF16, tag="qm", bufs=3)
                nc.gpsimd.dma_start(qmt, QMd[b:b + 1, :])
                kbt = psm.tile([1, KTL], BF16, tag="kb", bufs=3)
                nc.gpsimd.dma_start(kbt, KBd[b:b + 1, :])
                QTsA = pin.tile([P, NDT // 2, QTL], BF16, tag="qta",
                                name=f"qtsa{b}")
                nc.sync.dma_start(QTsA.rearrange("p t q -> p (t q)"),
                                  QTAd[b])
                KTsA = pin.tile([P, NDT // 2, KTL], BF16, tag="kta",
                                name=f"ktsa{b}")
                nc.sync.dma_start(KTsA.rearrange("p t k -> p (t k)"),
                                  KTAd[b])
                QTsB = pin.tile([P, NDT // 2, QTL], BF16, tag="qtb",
                                name=f"qtsb{b}")
                nc.gpsimd.dma_start(QTsB.rearrange("p t q -> p (t q)"),
                                    QTBd[b])
                KTsB = pin.tile([P, NDT // 2, KTL], BF16, tag="ktb",
                                name=f"ktsb{b}")
                nc.sync.dma_start(KTsB.rearrange("p t k -> p (t k)"),
                                  KTBd[b])
                # scalar queue carries ZERO dma dispatches: ACT compute can
                # never stall behind a backed-up DMA ring
                V8 = pin.tile([P, NKT // 2, D, 2], F8, tag="v",
                              name=f"v8{b}")
                nc.gpsimd.dma_start(
                    V8.rearrange("p t d i -> p (t d i)"), Vd[b])
                Qf = pin.tile([P, NQT, D], BF16, tag="q", name=f"qf{b}")
                nc.gpsimd.dma_start(Qf.rearrange("p t d -> p (t d)"), Qd[b])
                return dict(QTsA=QTsA, QTsB=QTsB, KTsA=KTsA, KTsB=KTsB,
                            Qf=Qf, V8=V8, qmt=qmt, kbt=kbt)

            ins = {0: load_batch(0)}

            # PE warmup while batch-0 inputs stream in: keeps HAM busy and
            # flips it to 8/8 right as the first S matmuls arrive
            wps = psF.tile([P, 512], F32, tag="f", name="warm")
            for _ in range(16):
                nc.tensor.matmul(wps, wz, wz512, start=True, stop=True)

            state = {}

            def ln_scalars(mv, tagp, nqt=NQT, iters=0):
                # r = rsqrt(var+eps) = exp(-0.5*ln(var+eps)) on ACT (the
                # natural_log_exp table holds both -> no table reloads);
                # nm = -mu*r as ONE gpsimd STT (idle engine, so the serial
                # LN chain is not diluted by DVE/ACT bulk work).
                lnv = psm.tile([P, nqt], F32, tag=f"{tagp}lv")
                nc.scalar.activation(lnv, mv[:, :, 1], AF.Ln,
                                     bias=epsb[:, :])
                r = psm.tile([P, nqt], F32, tag=f"{tagp}r")
                nc.scalar.activation(r, lnv, AF.Exp,
                                     scale=nhalfb[:, :])
                nm = psm.tile([P, nqt], F32, tag=f"{tagp}nm")
                nc.vector.scalar_tensor_tensor(
                    nm, mv[:, :, 0], -1.0, r, op0=ALU.mult, op1=ALU.mult)
                return r, nm

            def attn(b):
                t = ins[b]
                QTh = (t["QTsA"], t["QTsB"])
                KTh = (t["KTsA"], t["KTsB"])
                V8, Qf = t["V8"], t["Qf"]
                qmt, kbt = t["qmt"], t["kbt"]
                last = b == BL - 1

                # --- S^T = K^T-tiles . Q'T (+ kb x qm), 4 psum banks ---
                Sps = [psS.tile([P, QTL], F32, tag=f"s{kt}",
                                name=f"sps{kt}_{b}")
                       for kt in range(NKT)]
                for dt in range(NDT):
                    for kt in range(NKT):
                        nc.tensor.matmul(
                            Sps[kt],
                            KTh[dt // 4][:, dt % 4, kt * P:(kt + 1) * P],
                            QTh[dt // 4][:, dt % 4, :],
                            start=(dt == 0), stop=False)
                for kt in range(NKT):
                    nc.tensor.matmul(
                        Sps[kt], kbt[:, kt * P:(kt + 1) * P], qmt[:, :],
                        start=False, stop=True)

                # --- E = exp(S/32 - ln16), fp8e4 ---
                E = peh.tile([P, NKT, QTL], F8, tag="e", name=f"e{b}")
                for kt in range(NKT):
                    nc.scalar.activation(
                        E[:, kt, :], Sps[kt], AF.Exp,
                        bias=nln16b[:, :], scale=s32b[:, :])

                # --- rowsum^T per q-tile (E_tile^T @ ones), recip ---
                rsps = psU.tile([P, NQT], F32, tag="u", name=f"rsps{b}")
                for qt in range(NQT):
                    for kt in range(NKT):
                        nc.tensor.matmul(
                            rsps[:, qt:qt + 1],
                            E[:, kt, qt * P:(qt + 1) * P],
                            onesb[:, :],
                            start=(kt == 0), stop=(kt == NKT - 1))
                recT = psm.tile([P, NQT], F32, tag="recT")
                nc.vector.reciprocal(recT, rsps)

                # --- U = E^T V (fp8 DoubleRow) + drain, LN1 stats ---
                qres_l = []
                st1_l = []
                for qt in range(NQT):
                    qres = pst.tile([P, D], BF16, tag="qres", bufs=4,
                                    name=f"qres{qt}_{b}")
                    qres_l.append(qres)
                    st1 = psm.tile([P, NCH, 6], F32, tag=f"st1{qt % 2}")
                    st1_l.append(st1)
                    for ch in range(NCH):
                        Ups = psU.tile([P, 512], F32, tag="u")
                        for k2 in range(NKT // 2):
                            rhs = V8[:, k2, ch * 512:(ch + 1) * 512, :]
                            nc.tensor.matmul(
                                Ups,
                                E[:, 2 * k2:2 * k2 + 2,
                                  qt * P:(qt + 1) * P],
                                rhs.rearrange("p d i -> p i d"),
                                start=(k2 == 0), stop=(k2 == NKT // 2 - 1),
                                perf_mode=DR)
                        qch = qres[:, ch * 512:(ch + 1) * 512]
                        nc.vector.scalar_tensor_tensor(
                            qch, Ups, recT[:, qt:qt + 1],
                            Qf[:, qt, ch * 512:(ch + 1) * 512],
                            op0=ALU.mult, op1=ALU.add)
                        nc.vector.bn_stats(st1[:, ch, :], qch)

                y = pmid.tile([P, NQT, D], BF16, tag="y", name=f"y{b}")
                YT4 = pmid.tile([P, NQT, NDT, P], BF16, tag="yt4",
                                name=f"yt4{b}")

                # per-qt chain (aggr->ln->y->transpose): the first yT
                # blocks FFN1, so keep the per-qt critical path short
                for qt in range(NQT):
                    mq = psm.tile([P, 1, 2], F32, tag=f"mv1q{qt % 2}")
                    nc.vector.bn_aggr(mq[:, 0, :], st1_l[qt])
                    r1s, nm1s = ln_scalars(mq, f"l1q{qt % 2}", nqt=1)
                    nc.scalar.activation(
                        y[:, qt, :], qres_l[qt], AF.Identity,
                        bias=nm1s[:, 0:1], scale=r1s[:, 0:1])
                    if apply1:
                        yf = y[:, qt, :]
                        nc.vector.tensor_mul(yf, yf, g1t)
                        nc.vector.tensor_add(yf, yf, b1t)
                    nc.sync.dma_start_transpose(YT4[:, qt, :, :],
                                                y[:, qt, :])
                state[b] = dict(y=y, YT4=YT4)

            def ffn1(b):
                YT4 = state[b]["YT4"]
                # --- FFN1: H^T = relu(W1T . yT + b1) ---
                HT = peh.tile([P, NDT, QTL], BF16, tag="ht", name=f"ht{b}")
                for ot in range(NDT):
                    Hps = psF.tile([P, QTL], F32, tag="f")
                    for dt in range(NDT):
                        nc.tensor.matmul(
                            Hps,
                            W1Ct[ot][:, dt, :],
                            YT4[:, :, dt, :],
                            start=(dt == 0), stop=(dt == NDT - 1))
                    nc.scalar.activation(HT[:, ot, :], Hps, AF.Relu,
                                         bias=b1b[:, :])
                state[b]["HT"] = HT

            def ffn2(b):
                y, HT = state[b]["y"], state[b]["HT"]
                last = b == BL - 1

                def fin2(qt, r2s, nm2, j=0):
                    stg = pst.tile([P, D], F16, tag="stg", bufs=2,
                                   name=f"stg{qt % 2}_{b}")
                    nc.scalar.activation(
                        stg, r2_l[qt], AF.Identity,
                        bias=nm2[:, j:j + 1], scale=r2s[:, j:j + 1])
                    if apply2:
                        nc.vector.tensor_mul(stg, stg, g2t)
                        nc.vector.tensor_add(stg, stg, b2t)
                    if last:
                        nc.sync.dma_start(OUTd[b][qt][:, :D // 2],
                                          stg[:, :D // 2])
                        nc.scalar.dma_start(OUTd[b][qt][:, D // 2:],
                                            stg[:, D // 2:])
                    else:
                        nc.sync.dma_start(OUTd[b][qt], stg)

                if not last:
                    mv2 = psm.tile([P, NQT, 2], F32, tag="mv2")
                r2_l = []
                for qt in range(NQT):
                    r2 = pst.tile([P, D], F32, tag="r2", bufs=4,
                                  name=f"r2_{qt}_{b}")
                    r2_l.append(r2)
                    st2 = psm.tile([P, NCH, 6], F32, tag="st2")
                    for ch in range(NCH):
                        Zps = psS.tile([P, 512], F32,
                                       tag=f"s{(qt * NCH + ch) % 4}",
                                       name=f"zps{qt}{ch}_{b}")
                        for ot in range(NDT):
                            nc.tensor.matmul(
                                Zps,
                                HT[:, ot, qt * P:(qt + 1) * P],
                                W2Ct[ch][:, ot, :],
                                start=(ot == 0), stop=(ot == NDT - 1))
                        rch = r2[:, ch * 512:(ch + 1) * 512]
                        nc.vector.scalar_tensor_tensor(
                            rch, Zps, 1.0,
                            y[:, qt, ch * 512:(ch + 1) * 512],
                            op0=ALU.mult, op1=ALU.add)
                        nc.vector.bn_stats(st2[:, ch, :], rch)
                    if last:
                        mvq = psm.tile([P, 1, 2], F32, tag=f"mv2q{qt % 2}")
                        nc.vector.bn_aggr(mvq[:, 0, :], st2)
                        r2s, nm2 = ln_scalars(mvq, f"l2q{qt % 2}", nqt=1,
                                              iters=2)
                        fin2(qt, r2s, nm2)
                    else:
                        nc.vector.bn_aggr(mv2[:, qt, :], st2)

                if not last:
                    r2s, nm2 = ln_scalars(mv2, "l2", iters=2)
                    for qt in range(NQT):
                        fin2(qt, r2s, nm2, j=qt)

            assert BL == 4
            for ot in range(NDT):
                nc.sync.dma_start(
                    W1Ct[ot].rearrange("p t c -> p (t c)"), W1Cd[ot])
            ins[1] = load_batch(1)
            attn(0)
            for ch in range(NCH):
                nc.sync.dma_start(
                    W2Ct[ch].rearrange("p t c -> p (t c)"), W2Cd[ch])
            ins[2] = load_batch(2)
            ffn1(0)
            attn(1)
            ffn2(0)
            ins[3] = load_batch(3)
            attn(2)
            ffn1(1)
            ffn2(1)
            attn(3)
            ffn1(2)
            ffn2(2)
            ffn1(3)
            ffn2(3)

    nc.finalize()
    return nc


def _prepare(Q, K, V, Q_lengths, K_lengths, W1, b1, W2, b2,
             ln1_g, ln1_b, ln2_g, ln2_b):
    Q = np.asarray(Q, dtype=np.float32)
    K = np.asarray(K, dtype=np.float32)
    V = np.asarray(V, dtype=np.float32)
    W1 = np.asarray(W1, dtype=np.float32)
    W2 = np.asarray(W2, dtype=np.float32)
    qlen = np.asarray(Q_lengths).astype(np.int64)
    klen = np.asarray(K_lengths).astype(np.int64)
    g1 = np.asarray(ln1_g, dtype=np.float32)
    b1v = np.asarray(ln1_b, dtype=np.float32)
    g2 = np.asarray(ln2_g, dtype=np.float32)
    b2v = np.asarray(ln2_b, dtype=np.float32)
    b1f = float(np.asarray(b1, dtype=np.float32).reshape(-1)[0])
    # b2 cancels exactly inside LN2.

    apply1 = not (np.all(g1 == 1.0) and np.all(b1v == 0.0))
    apply2 = not (np.all(g2 == 1.0) and np.all(b2v == 0.0))

    def tile_rows(x):
        # [B, R, C] -> [B, P, (R/P)*C] in the "(t p)" SBUF tile layout
        Bn, R, C = x.shape
        return np.ascontiguousarray(
            x.reshape(Bn, R // P, P, C).transpose(0, 2, 1, 3).reshape(
                Bn, P, (R // P) * C))

    qmask = (np.arange(QTL)[None, :] < qlen[:, None])
    QT = np.ascontiguousarray(Q.transpose(0, 2, 1)).astype(NP_BF16)
    QT = QT * qmask[:, None, :].astype(NP_BF16)
    QT = tile_rows(QT)
    KT = tile_rows(np.ascontiguousarray(K.transpose(0, 2, 1)).astype(NP_BF16))
    HQK = QT.shape[2] // 2
    QTA = np.ascontiguousarray(QT[:, :, :HQK])
    QTB = np.ascontiguousarray(QT[:, :, HQK:])
    KTA = np.ascontiguousarray(KT[:, :, :HQK])
    KTB = np.ascontiguousarray(KT[:, :, HQK:])
    # DoubleRow rhs wants the k-pair adjacent in memory: [B,P,kt2,d,i]
    Vt = V.astype(NP_F8).reshape(B, NKT // 2, 2, P, D)      # [B,kt2,i,p,d]
    V8 = np.ascontiguousarray(
        Vt.transpose(0, 3, 1, 4, 2)).reshape(B, P, NKT * D)  # p,kt2,d,i
    Qb = tile_rows(Q.astype(NP_BF16))
    qmb = qmask.astype(NP_BF16)
    kbb = np.where(np.arange(KTL)[None, :] < klen[:, None], 0.0, NEG
                   ).astype(NP_BF16)
    W1T = tile_rows(np.ascontiguousarray(W1.T).astype(NP_BF16)[None])[0]
    W2T = tile_rows(np.ascontiguousarray(W2.T).astype(NP_BF16)[None])[0]
    # JIT chunk layouts: W1C[ot] = [P, dt, 128], W2C[ch] = [P, ot, 512]
    W1C = np.ascontiguousarray(
        W1T.reshape(P, NDT, NDT, P).transpose(2, 0, 1, 3)).reshape(
        NDT, P, NDT * P)
    W2C = np.ascontiguousarray(
        W2T.reshape(P, NDT, NCH, 512).transpose(2, 0, 1, 3)).reshape(
        NCH, P, NDT * 512)

    nc = _build(apply1, apply2, b1f)

    in_maps = []
    for c in range(NCORES):
        s = slice(c * BL, (c + 1) * BL)
        m = {
            "QTAp": QTA[s], "QTBp": QTB[s], "KTAp": KTA[s],
            "KTBp": KTB[s], "V8p": V8[s], "Qp": Qb[s],
            "QMp": qmb[s], "KBp": kbb[s],
            "W1Cp": W1C, "W2Cp": W2C,
        }
        if apply1:
            m["G1p"] = g1
            m["B1p"] = b1v
        if apply2:
            m["G2p"] = g2
            m["B2p"] = b2v
        in_maps.append(m)

    return nc, in_maps


def kernel(**inputs):
    nc, in_maps = _prepare(**inputs)
    res = run_bass_kernel_spmd(nc, in_maps, list(range(NCORES)))
    out = np.concatenate(
        [res.results[c]["OUTp"].reshape(BL, QTL, D) for c in range(NCORES)],
        axis=0)
    return out.astype(np.float32)


# revision 29
# speedup vs baseline: 1.1914x; 1.1914x over previous
"""Trainium2 Bass kernel: single-head attention + FFN transformer block.

Matmuls in bf16 except U (fp8e4 DoubleRow); S^T layout (math per batch b):
  S^T[k,q] = sum_d K[d,k] Q'[d,q] + kb[k]*qm[q]   (Q' = Q^T, invalid-q cols
                                                   zeroed host-side; kb = 0
                                                   valid / -1e30 masked)
  E[k,q]   = exp(S^T/32 - ln16)     ACT, fp8e4 out. Valid rows: softmax numer
                                    scaled by 1/16 (cancels in the ratio);
                                    invalid q: E=1/16 uniform over ALL k ->
                                    att = mean(V), matching the reference.
  rowsum   = ones^T E (PE),  recip via DVE, scattered to [q-part] layout.
  U[q,d]   = E^T_tile V      fp8 DoubleRow, K=256/MM. V is pair-interleaved
                             host-side (k-pair adjacent bytes) so the moving
                             operand streams N columns, not 2N.
  qres     = att + Q         fused on DVE (scalar_tensor_tensor), bf16
  y        = LN1(qres)       per-qt chain: bn_stats/aggr on DVE, then
                             rsqrt = exp(-0.5*ln(var+eps)) on ACT (the
                             natural_log_exp table holds both; the table-set
                             picker is pinned below so the whole kernel does
                             ONE ACT_TABLE_LOAD), nm = -mu*r as one DVE STT
  yT       = XBAR dma transpose per qt, ALL on the sync queue (concurrent
             transposes on two HWDGE queues race on the crossbar and corrupt
             blocks; the crossbar is also slow ~85GB/s, so only y uses it)
  H^T[o,q] = relu(W1T yT + b1)   bf16, W1 streamed as 8 JIT column chunks
  Z[q,d]   = H^T W2T;   out = LN2(y + Z)  (b2 cancels inside LN2)

Sharding: data-parallel, 4 batches per core on 8 cores.

Emission order (= scheduler priority): attn(b+1) is hoisted ahead of
ffn1(b) for b>=1 so its S/U matmuls fill the PE while ffn1(b) waits on the
y(b)->yT chain; attn(1) stays after ffn1(0) because batch 1's inputs are
still on the wire at that point. The last batch runs FFN2's LN2 per-qt to
shorten the drain tail.

Queues (engine FIFO head-of-line blocking is the enemy):
  sync   = all XBAR transposes + output stores + QT/KT bulk + W chunks
           (no compute engine -> nothing latency-critical stalls)
  scalar = ACT compute ONLY, zero DMA dispatches (a paced dispatch on a
           backed-up ring blocks every ACT op behind it in the FIFO)
  gpsimd = tiny mask loads + QTB/V8/Qf bulk (SWDGE, ~105GB/s)

PSUM: 4 S banks (reused by FFN2's Zps) + 2 U + 2 FFN1 = 8.
"""

import sys

sys.path.insert(0, "/opt/trn_rl_repo")

import numpy as np
import ml_dtypes

import concourse.bass as bass
import concourse.bacc as bacc
import concourse.mybir as mybir
from concourse import tile
from concourse.tile import add_dep_helper
from concourse.bass_utils import run_bass_kernel_spmd

# Force every activation this kernel uses (Exp, Ln, Relu, Identity) to
# resolve to the one table set that contains them all; the default set
# picker bounces between exp-only and ln-only sets, costing a ~1.3us
# ACT_TABLE_LOAD per LayerNorm rsqrt (39 loads = 50us of ACT time).
import concourse.hw_specs as _hw_specs

_ORIG_GAT = _hw_specs.get_activation_tables
_SHARED_SET = "natural_log_exp_and_others"


def _gat_pinned(module_arch):
    tabs = _ORIG_GAT(module_arch)
    af = mybir.ActivationFunctionType
    ours = {af.Exp, af.Ln, af.Relu, af.Identity, af.Copy}
    return {
        name: (funcs if name == _SHARED_SET else funcs - ours)
        for name, funcs in tabs.items()
    }


_hw_specs.get_activation_tables = _gat_pinned
bacc.get_activation_tables = _gat_pinned

B, QTL, KTL, D = 32, 512, 512, 1024
NCORES = 8
BL = B // NCORES
P = 128
NQT = QTL // P   # 4
NKT = KTL // P   # 4
NDT = D // P     # 8
NCH = 2          # 512-wide psum chunks per 1024
EPS = 1e-5
NEG = -1.0e30
LN16 = 2.772588722239781

F32 = mybir.dt.float32
F16 = mybir.dt.float16
BF16 = mybir.dt.bfloat16
F8 = mybir.dt.float8e4
AF = mybir.ActivationFunctionType
ALU = mybir.AluOpType
DR = mybir.MatmulPerfMode.DoubleRow
NP_BF16 = ml_dtypes.bfloat16
NP_F8 = ml_dtypes.float8_e4m3


def _build(apply1: bool, apply2: bool, b1f: float):
    nc = bacc.Bacc(None, target_bir_lowering=False)

    HQK = (NDT // 2) * KTL
    QTAd = nc.dram_tensor("QTAp", [BL, P, HQK], BF16, kind="ExternalInput")
    QTBd = nc.dram_tensor("QTBp", [BL, P, HQK], BF16, kind="ExternalInput")
    KTAd = nc.dram_tensor("KTAp", [BL, P, HQK], BF16, kind="ExternalInput")
    KTBd = nc.dram_tensor("KTBp", [BL, P, HQK], BF16, kind="ExternalInput")
    Vd = nc.dram_tensor("V8p", [BL, P, NKT * D], F8, kind="ExternalInput")
    Qd = nc.dram_tensor("Qp", [BL, P, NQT * D], BF16, kind="ExternalInput")
    QMd = nc.dram_tensor("QMp", [BL, QTL], BF16, kind="ExternalInput")
    KBd = nc.dram_tensor("KBp", [BL, KTL], BF16, kind="ExternalInput")
    W1Cd = nc.dram_tensor("W1Cp", [NDT, P, NDT * P], BF16,
                          kind="ExternalInput")
    W2Cd = nc.dram_tensor("W2Cp", [NCH, P, NDT * 512], BF16,
                          kind="ExternalInput")
    if apply1:
        G1d = nc.dram_tensor("G1p", [D], F32, kind="ExternalInput")
        B1d = nc.dram_tensor("B1p", [D], F32, kind="ExternalInput")
    if apply2:
        G2d = nc.dram_tensor("G2p", [D], F32, kind="ExternalInput")
        B2d = nc.dram_tensor("B2p", [D], F32, kind="ExternalInput")
    OUTd = nc.dram_tensor("OUTp", [BL, NQT, P, D], F16, kind="ExternalOutput")

    with tile.TileContext(nc) as tc:
        with (
            tc.tile_pool(name="const", bufs=1) as pc,
            tc.tile_pool(name="wts", bufs=1) as pw,
            tc.tile_pool(name="qkin", bufs=2) as pin,
            tc.tile_pool(name="mid", bufs=3) as pmid,
            tc.tile_pool(name="eh", bufs=1) as peh,
            tc.tile_pool(name="stream", bufs=2) as pst,
            tc.tile_pool(name="small", bufs=2) as psm,
            tc.tile_pool(name="psS", bufs=1, space="PSUM") as psS,
            tc.tile_pool(name="psU", bufs=2, space="PSUM") as psU,
            tc.tile_pool(name="psF", bufs=2, space="PSUM") as psF,
        ):
            # ---- constants / warmup ----
            wz = pc.tile([P, P], BF16)
            nc.vector.memset(wz, 0.0)
            wz512 = pc.tile([P, 512], BF16)
            nc.vector.memset(wz512, 0.0)
            onesb = pc.tile([P, 1], F8)
            nc.vector.memset(onesb, 1.0)
            nln16b = pc.tile([P, 1], F32)
            nc.vector.memset(nln16b, -LN16)
            s32b = pc.tile([P, 1], F32)
            nc.vector.memset(s32b, 1.0 / 32.0)
            b1b = pc.tile([P, 1], F32)
            nc.vector.memset(b1b, b1f)
            epsb = pc.tile([P, 1], F32)
            nc.vector.memset(epsb, EPS)
            nhalfb = pc.tile([P, 1], F32)
            nc.vector.memset(nhalfb, -0.5)
            if apply1:
                g1t = pc.tile([P, D], F32)
                nc.gpsimd.dma_start(
                    g1t, bass.AP(tensor=G1d, offset=0, ap=[[0, P], [1, D]]))
                b1t = pc.tile([P, D], F32)
                nc.gpsimd.dma_start(
                    b1t, bass.AP(tensor=B1d, offset=0, ap=[[0, P], [1, D]]))
            if apply2:
                g2t = pc.tile([P, D], F32)
                nc.gpsimd.dma_start(
                    g2t, bass.AP(tensor=G2d, offset=0, ap=[[0, P], [1, D]]))
                b2t = pc.tile([P, D], F32)
                nc.gpsimd.dma_start(
                    b2t, bass.AP(tensor=B2d, offset=0, ap=[[0, P], [1, D]]))

            W1Ct = [pw.tile([P, NDT, P], BF16, name=f"w1c{ot}")
                    for ot in range(NDT)]
            W2Ct = [pw.tile([P, NDT, 512], BF16, name=f"w2c{ch}")
                    for ch in range(NCH)]

            def load_batch(b):
                # tiny mask loads first (they gate the S-bias matmuls),
                # then bulk; sync stays free for transposes + stores.
                qmt = psm.tile([1, QTL], BF16, tag="qm", bufs=3)
                nc.gpsimd.dma_start(qmt, QMd[b:b + 1, :])
                kbt = psm.tile([1, KTL], BF16, tag="kb", bufs=3)
                nc.gpsimd.dma_start(kbt, KBd[b:b + 1, :])
                QTsA = pin.tile([P, NDT // 2, QTL], BF16, tag="qta",
                                name=f"qtsa{b}")
                nc.sync.dma_start(QTsA.rearrange("p t q -> p (t q)"),
                                  QTAd[b])
                KTsA = pin.tile([P, NDT // 2, KTL], BF16, tag="kta",
                                name=f"ktsa{b}")
                nc.sync.dma_start(KTsA.rearrange("p t k -> p (t k)"),
                                  KTAd[b])
                QTsB = pin.tile([P, NDT // 2, QTL], BF16, tag="qtb",
                                name=f"qtsb{b}")
                nc.gpsimd.dma_start(QTsB.rearrange("p t q -> p (t q)"),
                                    QTBd[b])
                KTsB = pin.tile([P, NDT // 2, KTL], BF16, tag="ktb",
                                name=f"ktsb{b}")
                nc.sync.dma_start(KTsB.rearrange("p t k -> p (t k)"),
                                  KTBd[b])
                # scalar queue carries ZERO dma dispatches: ACT compute can
                # never stall behind a backed-up DMA ring
                V8 = pin.tile([P, NKT // 2, D, 2], F8, tag="v",
                              name=f"v8{b}")
                nc.gpsimd.dma_start(
                    V8.rearrange("p t d i -> p (t d i)"), Vd[b])
                Qf = pin.tile([P, NQT, D], BF16, tag="q", name=f"qf{b}")
                nc.gpsimd.dma_start(Qf.rearrange("p t d -> p (t d)"), Qd[b])
                return dict(QTsA=QTsA, QTsB=QTsB, KTsA=KTsA, KTsB=KTsB,
                            Qf=Qf, V8=V8, qmt=qmt, kbt=kbt)

            ins = {0: load_batch(0)}

            # PE warmup while batch-0 inputs stream in: keeps HAM busy and
            # flips it to 8/8 right as the first S matmuls arrive
            wps = psF.tile([P, 512], F32, tag="f", name="warm")
            for _ in range(16):
                nc.tensor.matmul(wps, wz, wz512, start=True, stop=True)

            state = {}

            def ln_scalars(mv, tagp, nqt=NQT, iters=0):
                # r = rsqrt(var+eps) = exp(-0.5*ln(var+eps)) on ACT (the
                # natural_log_exp table holds both -> no table reloads);
                # nm = -mu*r as ONE gpsimd STT (idle engine, so the serial
                # LN chain is not diluted by DVE/ACT bulk work).
                lnv = psm.tile([P, nqt], F32, tag=f"{tagp}lv")
                nc.scalar.activation(lnv, mv[:, :, 1], AF.Ln,
                                     bias=epsb[:, :])
                r = psm.tile([P, nqt], F32, tag=f"{tagp}r")
                nc.scalar.activation(r, lnv, AF.Exp,
                                     scale=nhalfb[:, :])
                nm = psm.tile([P, nqt], F32, tag=f"{tagp}nm")
                nc.vector.scalar_tensor_tensor(
                    nm, mv[:, :, 0], -1.0, r, op0=ALU.mult, op1=ALU.mult)
                return r, nm

            def attn(b):
                t = ins[b]
                QTh = (t["QTsA"], t["QTsB"])
                KTh = (t["KTsA"], t["KTsB"])
                V8, Qf = t["V8"], t["Qf"]
                qmt, kbt = t["qmt"], t["kbt"]
                last = b == BL - 1

                # --- S^T = K^T-tiles . Q'T (+ kb x qm), 4 psum banks ---
                Sps = [psS.tile([P, QTL], F32, tag=f"s{kt}",
                                name=f"sps{kt}_{b}")
                       for kt in range(NKT)]
                for dt in range(NDT):
                    for kt in range(NKT):
                        nc.tensor.matmul(
                            Sps[kt],
                            KTh[dt // 4][:, dt % 4, kt * P:(kt + 1) * P],
                            QTh[dt // 4][:, dt % 4, :],
                            start=(dt == 0), stop=False)
                for kt in range(NKT):
                    nc.tensor.matmul(
                        Sps[kt], kbt[:, kt * P:(kt + 1) * P], qmt[:, :],
                        start=False, stop=True)

                # --- E = exp(S/32 - ln16), fp8e4 ---
                E = peh.tile([P, NKT, QTL], F8, tag="e", name=f"e{b}")
                for kt in range(NKT):
                    nc.scalar.activation(
                        E[:, kt, :], Sps[kt], AF.Exp,
                        bias=nln16b[:, :], scale=s32b[:, :])

                # --- rowsum^T per q-tile (E_tile^T @ ones), recip ---
                rsps = psU.tile([P, NQT], F32, tag="u", name=f"rsps{b}")
                for qt in range(NQT):
                    for kt in range(NKT):
                        nc.tensor.matmul(
                            rsps[:, qt:qt + 1],
                            E[:, kt, qt * P:(qt + 1) * P],
                            onesb[:, :],
                            start=(kt == 0), stop=(kt == NKT - 1))
                recT = psm.tile([P, NQT], F32, tag="recT")
                nc.vector.reciprocal(recT, rsps)

                # --- U = E^T V (fp8 DoubleRow) + drain, LN1 stats ---
                qres_l = []
                st1_l = []
                for qt in range(NQT):
                    qres = pst.tile([P, D], BF16, tag="qres", bufs=4,
                                    name=f"qres{qt}_{b}")
                    qres_l.append(qres)
                    st1 = psm.tile([P, NCH, 6], F32, tag=f"st1{qt % 2}")
                    st1_l.append(st1)
                    for ch in range(NCH):
                        Ups = psU.tile([P, 512], F32, tag="u")
                        for k2 in range(NKT // 2):
                            rhs = V8[:, k2, ch * 512:(ch + 1) * 512, :]
                            nc.tensor.matmul(
                                Ups,
                                E[:, 2 * k2:2 * k2 + 2,
                                  qt * P:(qt + 1) * P],
                                rhs.rearrange("p d i -> p i d"),
                                start=(k2 == 0), stop=(k2 == NKT // 2 - 1),
                                perf_mode=DR)
                        qch = qres[:, ch * 512:(ch + 1) * 512]
                        nc.vector.scalar_tensor_tensor(
                            qch, Ups, recT[:, qt:qt + 1],
                            Qf[:, qt, ch * 512:(ch + 1) * 512],
                            op0=ALU.mult, op1=ALU.add)
                        nc.vector.bn_stats(st1[:, ch, :], qch)

                y = pmid.tile([P, NQT, D], BF16, tag="y", name=f"y{b}")
                YT4 = pmid.tile([P, NQT, NDT, P], BF16, tag="yt4",
                                name=f"yt4{b}")

                # per-qt chain (aggr->ln->y->transpose): the first yT
                # blocks FFN1, so keep the per-qt critical path short
                for qt in range(NQT):
                    mq = psm.tile([P, 1, 2], F32, tag=f"mv1q{qt % 2}")
                    nc.vector.bn_aggr(mq[:, 0, :], st1_l[qt])
                    r1s, nm1s = ln_scalars(mq, f"l1q{qt % 2}", nqt=1)
                    nc.scalar.activation(
                        y[:, qt, :], qres_l[qt], AF.Identity,
                        bias=nm1s[:, 0:1], scale=r1s[:, 0:1])
                    if apply1:
                        yf = y[:, qt, :]
                        nc.vector.tensor_mul(yf, yf, g1t)
                        nc.vector.tensor_add(yf, yf, b1t)
                    nc.sync.dma_start_transpose(YT4[:, qt, :, :],
                                                y[:, qt, :])
                state[b] = dict(y=y, YT4=YT4)

            def ffn1(b):
                YT4 = state[b]["YT4"]
                # --- FFN1: H^T = relu(W1T . yT + b1) ---
                HT = peh.tile([P, NDT, QTL], BF16, tag="ht", name=f"ht{b}")
                for ot in range(NDT):
                    Hps = psF.tile([P, QTL], F32, tag="f")
                    for dt in range(NDT):
                        nc.tensor.matmul(
                            Hps,
                            W1Ct[ot][:, dt, :],
                            YT4[:, :, dt, :],
                            start=(dt == 0), stop=(dt == NDT - 1))
                    nc.scalar.activation(HT[:, ot, :], Hps, AF.Relu,
                                         bias=b1b[:, :])
                state[b]["HT"] = HT

            def ffn2(b):
                y, HT = state[b]["y"], state[b]["HT"]
                last = b == BL - 1

                def fin2(qt, r2s, nm2, j=0):
                    stg = pst.tile([P, D], F16, tag="stg", bufs=2,
                                   name=f"stg{qt % 2}_{b}")
                    nc.scalar.activation(
                        stg, r2_l[qt], AF.Identity,
                        bias=nm2[:, j:j + 1], scale=r2s[:, j:j + 1])
                    if apply2:
                        nc.vector.tensor_mul(stg, stg, g2t)
                        nc.vector.tensor_add(stg, stg, b2t)
                    if last:
                        nc.sync.dma_start(OUTd[b][qt][:, :D // 2],
                                          stg[:, :D // 2])
                        nc.scalar.dma_start(OUTd[b][qt][:, D // 2:],
                                            stg[:, D // 2:])
                    else:
                        nc.sync.dma_start(OUTd[b][qt], stg)

                if not last:
                    mv2 = psm.tile([P, NQT, 2], F32, tag="mv2")
                r2_l = []
                for qt in range(NQT):
                    r2 = pst.tile([P, D], F32, tag="r2", bufs=4,
                                  name=f"r2_{qt}_{b}")
                    r2_l.append(r2)
                    st2 = psm.tile([P, NCH, 6], F32, tag="st2")
                    for ch in range(NCH):
                        Zps = psS.tile([P, 512], F32,
                                       tag=f"s{(qt * NCH + ch) % 4}",
                                       name=f"zps{qt}{ch}_{b}")
                        for ot in range(NDT):
                            nc.tensor.matmul(
                                Zps,
                                HT[:, ot, qt * P:(qt + 1) * P],
                                W2Ct[ch][:, ot, :],
                                start=(ot == 0), stop=(ot == NDT - 1))
                        rch = r2[:, ch * 512:(ch + 1) * 512]
                        nc.vector.scalar_tensor_tensor(
                            rch, Zps, 1.0,
                            y[:, qt, ch * 512:(ch + 1) * 512],
                            op0=ALU.mult, op1=ALU.add)
                        nc.vector.bn_stats(st2[:, ch, :], rch)
                    if last:
                        mvq = psm.tile([P, 1, 2], F32, tag=f"mv2q{qt % 2}")
                        nc.vector.bn_aggr(mvq[:, 0, :], st2)
                        r2s, nm2 = ln_scalars(mvq, f"l2q{qt % 2}", nqt=1,
                                              iters=2)
                        fin2(qt, r2s, nm2)
                    else:
                        nc.vector.bn_aggr(mv2[:, qt, :], st2)

                if not last:
                    r2s, nm2 = ln_scalars(mv2, "l2", iters=2)
                    for qt in range(NQT):
                        fin2(qt, r2s, nm2, j=qt)

            assert BL == 4
            for ot in range(NDT):
                nc.sync.dma_start(
                    W1Ct[ot].rearrange("p t c -> p (t c)"), W1Cd[ot])
            ins[1] = load_batch(1)
            attn(0)
            for ch in range(NCH):
                nc.sync.dma_start(
                    W2Ct[ch].rearrange("p t c -> p (t c)"), W2Cd[ch])
            ins[2] = load_batch(2)
            ffn1(0)
            attn(1)
            attn(2)
            ffn2(0)
            ins[3] = load_batch(3)
            ffn1(1)
            ffn2(1)
            attn(3)
            ffn1(2)
            ffn2(2)
            ffn1(3)
            ffn2(3)

    nc.finalize()
    return nc


def _prepare(Q, K, V, Q_lengths, K_lengths, W1, b1, W2, b2,
             ln1_g, ln1_b, ln2_g, ln2_b):
    Q = np.asarray(Q, dtype=np.float32)
    K = np.asarray(K, dtype=np.float32)
    V = np.asarray(V, dtype=np.float32)
    W1 = np.asarray(W1, dtype=np.float32)
    W2 = np.asarray(W2, dtype=np.float32)
    qlen = np.asarray(Q_lengths).astype(np.int64)
    klen = np.asarray(K_lengths).astype(np.int64)
    g1 = np.asarray(ln1_g, dtype=np.float32)
    b1v = np.asarray(ln1_b, dtype=np.float32)
    g2 = np.asarray(ln2_g, dtype=np.float32)
    b2v = np.asarray(ln2_b, dtype=np.float32)
    b1f = float(np.asarray(b1, dtype=np.float32).reshape(-1)[0])
    # b2 cancels exactly inside LN2.

    apply1 = not (np.all(g1 == 1.0) and np.all(b1v == 0.0))
    apply2 = not (np.all(g2 == 1.0) and np.all(b2v == 0.0))

    def tile_rows(x):
        # [B, R, C] -> [B, P, (R/P)*C] in the "(t p)" SBUF tile layout
        Bn, R, C = x.shape
        return np.ascontiguousarray(
            x.reshape(Bn, R // P, P, C).transpose(0, 2, 1, 3).reshape(
                Bn, P, (R // P) * C))

    qmask = (np.arange(QTL)[None, :] < qlen[:, None])
    QT = np.ascontiguousarray(Q.transpose(0, 2, 1)).astype(NP_BF16)
    QT = QT * qmask[:, None, :].astype(NP_BF16)
    QT = tile_rows(QT)
    KT = tile_rows(np.ascontiguousarray(K.transpose(0, 2, 1)).astype(NP_BF16))
    HQK = QT.shape[2] // 2
    QTA = np.ascontiguousarray(QT[:, :, :HQK])
    QTB = np.ascontiguousarray(QT[:, :, HQK:])
    KTA = np.ascontiguousarray(KT[:, :, :HQK])
    KTB = np.ascontiguousarray(KT[:, :, HQK:])
    # DoubleRow rhs wants the k-pair adjacent in memory: [B,P,kt2,d,i]
    Vt = V.astype(NP_F8).reshape(B, NKT // 2, 2, P, D)      # [B,kt2,i,p,d]
    V8 = np.ascontiguousarray(
        Vt.transpose(0, 3, 1, 4, 2)).reshape(B, P, NKT * D)  # p,kt2,d,i
    Qb = tile_rows(Q.astype(NP_BF16))
    qmb = qmask.astype(NP_BF16)
    kbb = np.where(np.arange(KTL)[None, :] < klen[:, None], 0.0, NEG
                   ).astype(NP_BF16)
    W1T = tile_rows(np.ascontiguousarray(W1.T).astype(NP_BF16)[None])[0]
    W2T = tile_rows(np.ascontiguousarray(W2.T).astype(NP_BF16)[None])[0]
    # JIT chunk layouts: W1C[ot] = [P, dt, 128], W2C[ch] = [P, ot, 512]
    W1C = np.ascontiguousarray(
        W1T.reshape(P, NDT, NDT, P).transpose(2, 0, 1, 3)).reshape(
        NDT, P, NDT * P)
    W2C = np.ascontiguousarray(
        W2T.reshape(P, NDT, NCH, 512).transpose(2, 0, 1, 3)).reshape(
        NCH, P, NDT * 512)

    nc = _build(apply1, apply2, b1f)

    in_maps = []
    for c in range(NCORES):
        s = slice(c * BL, (c + 1) * BL)
        m = {
            "QTAp": QTA[s], "QTBp": QTB[s], "KTAp": KTA[s],
            "KTBp": KTB[s], "V8p": V8[s], "Qp": Qb[s],
            "QMp": qmb[s], "KBp": kbb[s],
            "W1Cp": W1C, "W2Cp": W2C,
        }
        if apply1:
            m["G1p"] = g1
            m["B1p"] = b1v
        if apply2:
            m["G2p"] = g2
            m["B2p"] = b2v
        in_maps.append(m)

    return nc, in_maps


def kernel(**inputs):
    nc, in_maps = _prepare(**inputs)
    res = run_bass_kernel_spmd(nc, in_maps, list(range(NCORES)))
    out = np.concatenate(
        [res.results[c]["OUTp"].reshape(BL, QTL, D) for c in range(NCORES)],
        axis=0)
    return out.astype(np.float32)


# revision 30
# speedup vs baseline: 1.1956x; 1.0035x over previous
"""Trainium2 Bass kernel: single-head attention + FFN transformer block.

Matmuls in bf16 except U (fp8e4 DoubleRow); S^T layout (math per batch b):
  S^T[k,q] = sum_d K[d,k] Q'[d,q] + kb[k]*qm[q]   (Q' = Q^T, invalid-q cols
                                                   zeroed host-side; kb = 0
                                                   valid / -1e30 masked)
  E[k,q]   = exp(S^T/32 - ln16)     ACT, fp8e4 out. Valid rows: softmax numer
                                    scaled by 1/16 (cancels in the ratio);
                                    invalid q: E=1/16 uniform over ALL k ->
                                    att = mean(V), matching the reference.
  rowsum   = ones^T E (PE),  recip via DVE, scattered to [q-part] layout.
  U[q,d]   = E^T_tile V      fp8 DoubleRow, K=256/MM. V is pair-interleaved
                             host-side (k-pair adjacent bytes) so the moving
                             operand streams N columns, not 2N.
  qres     = att + Q         fused on DVE (scalar_tensor_tensor), bf16
  y        = LN1(qres)       per-qt chain: bn_stats/aggr on DVE, then
                             rsqrt = exp(-0.5*ln(var+eps)) on ACT (the
                             natural_log_exp table holds both; the table-set
                             picker is pinned below so the whole kernel does
                             ONE ACT_TABLE_LOAD), nm = -mu*r as one DVE STT
  yT       = XBAR dma transpose per qt, ALL on the sync queue (concurrent
             transposes on two HWDGE queues race on the crossbar and corrupt
             blocks; the crossbar is also slow ~85GB/s, so only y uses it)
  H^T[o,q] = relu(W1T yT + b1)   bf16, W1 streamed as 8 JIT column chunks
  Z[q,d]   = H^T W2T;   out = LN2(y + Z)  (b2 cancels inside LN2)

Sharding: data-parallel, 4 batches per core on 8 cores.

Emission order (= scheduler priority): attn(b+1) is hoisted ahead of
ffn1(b) for b>=1 so its S/U matmuls fill the PE while ffn1(b) waits on the
y(b)->yT chain; attn(1) stays after ffn1(0) because batch 1's inputs are
still on the wire at that point. The last batch runs FFN2's LN2 per-qt to
shorten the drain tail.

Queues (engine FIFO head-of-line blocking is the enemy):
  sync   = all XBAR transposes + output stores + QT/KT bulk + W chunks
           (no compute engine -> nothing latency-critical stalls)
  scalar = ACT compute ONLY, zero DMA dispatches (a paced dispatch on a
           backed-up ring blocks every ACT op behind it in the FIFO)
  gpsimd = tiny mask loads + QTB/V8/Qf bulk (SWDGE, ~105GB/s)

PSUM: 4 S banks (reused by FFN2's Zps) + 2 U + 2 FFN1 = 8.
"""

import sys

sys.path.insert(0, "/opt/trn_rl_repo")

import numpy as np
import ml_dtypes

import concourse.bass as bass
import concourse.bacc as bacc
import concourse.mybir as mybir
from concourse import tile
from concourse.tile import add_dep_helper
from concourse.bass_utils import run_bass_kernel_spmd

# Force every activation this kernel uses (Exp, Ln, Relu, Identity) to
# resolve to the one table set that contains them all; the default set
# picker bounces between exp-only and ln-only sets, costing a ~1.3us
# ACT_TABLE_LOAD per LayerNorm rsqrt (39 loads = 50us of ACT time).
import concourse.hw_specs as _hw_specs

_ORIG_GAT = _hw_specs.get_activation_tables
_SHARED_SET = "natural_log_exp_and_others"


def _gat_pinned(module_arch):
    tabs = _ORIG_GAT(module_arch)
    af = mybir.ActivationFunctionType
    ours = {af.Exp, af.Ln, af.Relu, af.Identity, af.Copy}
    return {
        name: (funcs if name == _SHARED_SET else funcs - ours)
        for name, funcs in tabs.items()
    }


_hw_specs.get_activation_tables = _gat_pinned
bacc.get_activation_tables = _gat_pinned

B, QTL, KTL, D = 32, 512, 512, 1024
NCORES = 8
BL = B // NCORES
P = 128
NQT = QTL // P   # 4
NKT = KTL // P   # 4
NDT = D // P     # 8
NCH = 2          # 512-wide psum chunks per 1024
EPS = 1e-5
NEG = -1.0e30
LN16 = 2.772588722239781

F32 = mybir.dt.float32
F16 = mybir.dt.float16
BF16 = mybir.dt.bfloat16
F8 = mybir.dt.float8e4
AF = mybir.ActivationFunctionType
ALU = mybir.AluOpType
DR = mybir.MatmulPerfMode.DoubleRow
NP_BF16 = ml_dtypes.bfloat16
NP_F8 = ml_dtypes.float8_e4m3


def _build(apply1: bool, apply2: bool, b1f: float):
    nc = bacc.Bacc(None, target_bir_lowering=False)

    HQK = (NDT // 2) * KTL
    QTAd = nc.dram_tensor("QTAp", [BL, P, HQK], BF16, kind="ExternalInput")
    QTBd = nc.dram_tensor("QTBp", [BL, P, HQK], BF16, kind="ExternalInput")
    KTAd = nc.dram_tensor("KTAp", [BL, P, HQK], BF16, kind="ExternalInput")
    KTBd = nc.dram_tensor("KTBp", [BL, P, HQK], BF16, kind="ExternalInput")
    Vd = nc.dram_tensor("V8p", [BL, P, NKT * D], F8, kind="ExternalInput")
    Qd = nc.dram_tensor("Qp", [BL, P, NQT * D], BF16, kind="ExternalInput")
    QMd = nc.dram_tensor("QMp", [BL, QTL], BF16, kind="ExternalInput")
    KBd = nc.dram_tensor("KBp", [BL, KTL], BF16, kind="ExternalInput")
    W1Cd = nc.dram_tensor("W1Cp", [NDT, P, NDT * P], BF16,
                          kind="ExternalInput")
    W2Cd = nc.dram_tensor("W2Cp", [NCH, P, NDT * 512], BF16,
                          kind="ExternalInput")
    if apply1:
        G1d = nc.dram_tensor("G1p", [D], F32, kind="ExternalInput")
        B1d = nc.dram_tensor("B1p", [D], F32, kind="ExternalInput")
    if apply2:
        G2d = nc.dram_tensor("G2p", [D], F32, kind="ExternalInput")
        B2d = nc.dram_tensor("B2p", [D], F32, kind="ExternalInput")
    OUTd = nc.dram_tensor("OUTp", [BL, NQT, P, D], F16, kind="ExternalOutput")

    with tile.TileContext(nc) as tc:
        with (
            tc.tile_pool(name="const", bufs=1) as pc,
            tc.tile_pool(name="wts", bufs=1) as pw,
            tc.tile_pool(name="qkin", bufs=2) as pin,
            tc.tile_pool(name="mid", bufs=3) as pmid,
            tc.tile_pool(name="eh", bufs=1) as peh,
            tc.tile_pool(name="stream", bufs=2) as pst,
            tc.tile_pool(name="small", bufs=2) as psm,
            tc.tile_pool(name="psS", bufs=1, space="PSUM") as psS,
            tc.tile_pool(name="psU", bufs=2, space="PSUM") as psU,
            tc.tile_pool(name="psF", bufs=2, space="PSUM") as psF,
        ):
            # ---- constants / warmup ----
            wz = pc.tile([P, P], BF16)
            nc.vector.memset(wz, 0.0)
            wz512 = pc.tile([P, 512], BF16)
            nc.vector.memset(wz512, 0.0)
            onesb = pc.tile([P, 1], F8)
            nc.vector.memset(onesb, 1.0)
            nln16b = pc.tile([P, 1], F32)
            nc.vector.memset(nln16b, -LN16)
            s32b = pc.tile([P, 1], F32)
            nc.vector.memset(s32b, 1.0 / 32.0)
            b1b = pc.tile([P, 1], F32)
            nc.vector.memset(b1b, b1f)
            epsb = pc.tile([P, 1], F32)
            nc.vector.memset(epsb, EPS)
            nhalfb = pc.tile([P, 1], F32)
            nc.vector.memset(nhalfb, -0.5)
            if apply1:
                g1t = pc.tile([P, D], F32)
                nc.gpsimd.dma_start(
                    g1t, bass.AP(tensor=G1d, offset=0, ap=[[0, P], [1, D]]))
                b1t = pc.tile([P, D], F32)
                nc.gpsimd.dma_start(
                    b1t, bass.AP(tensor=B1d, offset=0, ap=[[0, P], [1, D]]))
            if apply2:
                g2t = pc.tile([P, D], F32)
                nc.gpsimd.dma_start(
                    g2t, bass.AP(tensor=G2d, offset=0, ap=[[0, P], [1, D]]))
                b2t = pc.tile([P, D], F32)
                nc.gpsimd.dma_start(
                    b2t, bass.AP(tensor=B2d, offset=0, ap=[[0, P], [1, D]]))

            W1Ct = [pw.tile([P, NDT, P], BF16, name=f"w1c{ot}")
                    for ot in range(NDT)]
            W2Ct = [pw.tile([P, NDT, 512], BF16, name=f"w2c{ch}")
                    for ch in range(NCH)]

            def load_batch(b):
                # tiny mask loads first (they gate the S-bias matmuls),
                # then bulk; sync stays free for transposes + stores.
                qmt = psm.tile([1, QTL], BF16, tag="qm", bufs=3)
                nc.gpsimd.dma_start(qmt, QMd[b:b + 1, :])
                kbt = psm.tile([1, KTL], BF16, tag="kb", bufs=3)
                nc.gpsimd.dma_start(kbt, KBd[b:b + 1, :])
                QTsA = pin.tile([P, NDT // 2, QTL], BF16, tag="qta",
                                name=f"qtsa{b}")
                nc.sync.dma_start(QTsA.rearrange("p t q -> p (t q)"),
                                  QTAd[b])
                KTsA = pin.tile([P, NDT // 2, KTL], BF16, tag="kta",
                                name=f"ktsa{b}")
                nc.sync.dma_start(KTsA.rearrange("p t k -> p (t k)"),
                                  KTAd[b])
                QTsB = pin.tile([P, NDT // 2, QTL], BF16, tag="qtb",
                                name=f"qtsb{b}")
                nc.gpsimd.dma_start(QTsB.rearrange("p t q -> p (t q)"),
                                    QTBd[b])
                KTsB = pin.tile([P, NDT // 2, KTL], BF16, tag="ktb",
                                name=f"ktsb{b}")
                nc.sync.dma_start(KTsB.rearrange("p t k -> p (t k)"),
                                  KTBd[b])
                # scalar queue carries ZERO dma dispatches: ACT compute can
                # never stall behind a backed-up DMA ring
                V8 = pin.tile([P, NKT // 2, D, 2], F8, tag="v",
                              name=f"v8{b}")
                nc.gpsimd.dma_start(
                    V8.rearrange("p t d i -> p (t d i)"), Vd[b])
                Qf = pin.tile([P, NQT, D], BF16, tag="q", name=f"qf{b}")
                nc.gpsimd.dma_start(Qf.rearrange("p t d -> p (t d)"), Qd[b])
                return dict(QTsA=QTsA, QTsB=QTsB, KTsA=KTsA, KTsB=KTsB,
                            Qf=Qf, V8=V8, qmt=qmt, kbt=kbt)

            ins = {0: load_batch(0)}

            # PE warmup while batch-0 inputs stream in: keeps HAM busy and
            # flips it to 8/8 right as the first S matmuls arrive
            wps = psF.tile([P, 512], F32, tag="f", name="warm")
            for _ in range(16):
                nc.tensor.matmul(wps, wz, wz512, start=True, stop=True)

            state = {}

            def ln_scalars(mv, tagp, nqt=NQT, iters=0):
                # r = rsqrt(var+eps) = exp(-0.5*ln(var+eps)) on ACT (the
                # natural_log_exp table holds both -> no table reloads);
                # nm = -mu*r as ONE gpsimd STT (idle engine, so the serial
                # LN chain is not diluted by DVE/ACT bulk work).
                lnv = psm.tile([P, nqt], F32, tag=f"{tagp}lv")
                nc.scalar.activation(lnv, mv[:, :, 1], AF.Ln,
                                     bias=epsb[:, :])
                r = psm.tile([P, nqt], F32, tag=f"{tagp}r")
                nc.scalar.activation(r, lnv, AF.Exp,
                                     scale=nhalfb[:, :])
                nm = psm.tile([P, nqt], F32, tag=f"{tagp}nm")
                nc.vector.scalar_tensor_tensor(
                    nm, mv[:, :, 0], -1.0, r, op0=ALU.mult, op1=ALU.mult)
                return r, nm

            def attn(b):
                t = ins[b]
                QTh = (t["QTsA"], t["QTsB"])
                KTh = (t["KTsA"], t["KTsB"])
                V8, Qf = t["V8"], t["Qf"]
                qmt, kbt = t["qmt"], t["kbt"]
                last = b == BL - 1

                # --- S^T = K^T-tiles . Q'T (+ kb x qm), 4 psum banks ---
                Sps = [psS.tile([P, QTL], F32, tag=f"s{kt}",
                                name=f"sps{kt}_{b}")
                       for kt in range(NKT)]
                for dt in range(NDT):
                    for kt in range(NKT):
                        nc.tensor.matmul(
                            Sps[kt],
                            KTh[dt // 4][:, dt % 4, kt * P:(kt + 1) * P],
                            QTh[dt // 4][:, dt % 4, :],
                            start=(dt == 0), stop=False)
                for kt in range(NKT):
                    nc.tensor.matmul(
                        Sps[kt], kbt[:, kt * P:(kt + 1) * P], qmt[:, :],
                        start=False, stop=True)

                # --- E = exp(S/32 - ln16), fp8e4 ---
                E = peh.tile([P, NKT, QTL], F8, tag="e", name=f"e{b}")
                for kt in range(NKT):
                    nc.scalar.activation(
                        E[:, kt, :], Sps[kt], AF.Exp,
                        bias=nln16b[:, :], scale=s32b[:, :])

                # --- rowsum^T per q-tile (E_tile^T @ ones), recip ---
                rsps = psU.tile([P, NQT], F32, tag="u", name=f"rsps{b}")
                for qt in range(NQT):
                    for kt in range(NKT):
                        nc.tensor.matmul(
                            rsps[:, qt:qt + 1],
                            E[:, kt, qt * P:(qt + 1) * P],
                            onesb[:, :],
                            start=(kt == 0), stop=(kt == NKT - 1))
                recT = psm.tile([P, NQT], F32, tag="recT")
                nc.vector.reciprocal(recT, rsps)

                # --- U = E^T V (fp8 DoubleRow) + drain, LN1 stats ---
                qres_l = []
                st1_l = []
                for qt in range(NQT):
                    qres = pst.tile([P, D], BF16, tag="qres", bufs=4,
                                    name=f"qres{qt}_{b}")
                    qres_l.append(qres)
                    st1 = psm.tile([P, NCH, 6], F32, tag=f"st1{qt % 2}")
                    st1_l.append(st1)
                    for ch in range(NCH):
                        Ups = psU.tile([P, 512], F32, tag="u")
                        for k2 in range(NKT // 2):
                            rhs = V8[:, k2, ch * 512:(ch + 1) * 512, :]
                            nc.tensor.matmul(
                                Ups,
                                E[:, 2 * k2:2 * k2 + 2,
                                  qt * P:(qt + 1) * P],
                                rhs.rearrange("p d i -> p i d"),
                                start=(k2 == 0), stop=(k2 == NKT // 2 - 1),
                                perf_mode=DR)
                        qch = qres[:, ch * 512:(ch + 1) * 512]
                        nc.vector.scalar_tensor_tensor(
                            qch, Ups, recT[:, qt:qt + 1],
                            Qf[:, qt, ch * 512:(ch + 1) * 512],
                            op0=ALU.mult, op1=ALU.add)
                        nc.vector.bn_stats(st1[:, ch, :], qch)

                y = pmid.tile([P, NQT, D], BF16, tag="y", name=f"y{b}")
                YT4 = pmid.tile([P, NQT, NDT, P], BF16, tag="yt4",
                                name=f"yt4{b}")

                # per-qt chain (aggr->ln->y->transpose): the first yT
                # blocks FFN1, so keep the per-qt critical path short
                for qt in range(NQT):
                    mq = psm.tile([P, 1, 2], F32, tag=f"mv1q{qt % 2}")
                    nc.vector.bn_aggr(mq[:, 0, :], st1_l[qt])
                    r1s, nm1s = ln_scalars(mq, f"l1q{qt % 2}", nqt=1)
                    nc.scalar.activation(
                        y[:, qt, :], qres_l[qt], AF.Identity,
                        bias=nm1s[:, 0:1], scale=r1s[:, 0:1])
                    if apply1:
                        yf = y[:, qt, :]
                        nc.vector.tensor_mul(yf, yf, g1t)
                        nc.vector.tensor_add(yf, yf, b1t)
                    nc.sync.dma_start_transpose(YT4[:, qt, :, :],
                                                y[:, qt, :])
                state[b] = dict(y=y, YT4=YT4)

            def ffn1(b):
                YT4 = state[b]["YT4"]
                # --- FFN1: H^T = relu(W1T . yT + b1) ---
                HT = peh.tile([P, NDT, QTL], BF16, tag="ht", name=f"ht{b}")
                for ot in range(NDT):
                    Hps = psF.tile([P, QTL], F32, tag="f")
                    for dt in range(NDT):
                        nc.tensor.matmul(
                            Hps,
                            W1Ct[ot][:, dt, :],
                            YT4[:, :, dt, :],
                            start=(dt == 0), stop=(dt == NDT - 1))
                    nc.scalar.activation(HT[:, ot, :], Hps, AF.Relu,
                                         bias=b1b[:, :])
                state[b]["HT"] = HT

            def ffn2(b):
                y, HT = state[b]["y"], state[b]["HT"]
                last = b == BL - 1

                def fin2(qt, r2s, nm2, j=0):
                    stg = pst.tile([P, D], F16, tag="stg", bufs=2,
                                   name=f"stg{qt % 2}_{b}")
                    nc.scalar.activation(
                        stg, r2_l[qt], AF.Identity,
                        bias=nm2[:, j:j + 1], scale=r2s[:, j:j + 1])
                    if apply2:
                        nc.vector.tensor_mul(stg, stg, g2t)
                        nc.vector.tensor_add(stg, stg, b2t)
                    if last:
                        nc.sync.dma_start(OUTd[b][qt][:, :D // 2],
                                          stg[:, :D // 2])
                        nc.scalar.dma_start(OUTd[b][qt][:, D // 2:],
                                            stg[:, D // 2:])
                    else:
                        nc.sync.dma_start(OUTd[b][qt], stg)

                if not last:
                    mv2 = psm.tile([P, NQT, 2], F32, tag="mv2")
                r2_l = []
                for qt in range(NQT):
                    r2 = pst.tile([P, D], F32, tag="r2", bufs=4,
                                  name=f"r2_{qt}_{b}")
                    r2_l.append(r2)
                    st2 = psm.tile([P, NCH, 6], F32, tag="st2")
                    for ch in range(NCH):
                        Zps = psS.tile([P, 512], F32,
                                       tag=f"s{(qt * NCH + ch) % 4}",
                                       name=f"zps{qt}{ch}_{b}")
                        for ot in range(NDT):
                            nc.tensor.matmul(
                                Zps,
                                HT[:, ot, qt * P:(qt + 1) * P],
                                W2Ct[ch][:, ot, :],
                                start=(ot == 0), stop=(ot == NDT - 1))
                        rch = r2[:, ch * 512:(ch + 1) * 512]
                        nc.vector.scalar_tensor_tensor(
                            rch, Zps, 1.0,
                            y[:, qt, ch * 512:(ch + 1) * 512],
                            op0=ALU.mult, op1=ALU.add)
                        nc.vector.bn_stats(st2[:, ch, :], rch)
                    if last:
                        mvq = psm.tile([P, 1, 2], F32, tag=f"mv2q{qt % 2}")
                        nc.vector.bn_aggr(mvq[:, 0, :], st2)
                        r2s, nm2 = ln_scalars(mvq, f"l2q{qt % 2}", nqt=1,
                                              iters=2)
                        fin2(qt, r2s, nm2)
                    else:
                        nc.vector.bn_aggr(mv2[:, qt, :], st2)

                if not last:
                    r2s, nm2 = ln_scalars(mv2, "l2", iters=2)
                    for qt in range(NQT):
                        fin2(qt, r2s, nm2, j=qt)

            assert BL == 4
            ins[1] = load_batch(1)
            for ot in range(NDT):
                nc.sync.dma_start(
                    W1Ct[ot].rearrange("p t c -> p (t c)"), W1Cd[ot])
            attn(0)
            for ch in range(NCH):
                nc.sync.dma_start(
                    W2Ct[ch].rearrange("p t c -> p (t c)"), W2Cd[ch])
            ins[2] = load_batch(2)
            ffn1(0)
            attn(1)
            attn(2)
            ffn2(0)
            ins[3] = load_batch(3)
            ffn1(1)
            ffn2(1)
            attn(3)
            ffn1(2)
            ffn2(2)
            ffn1(3)
            ffn2(3)

    nc.finalize()
    return nc


def _prepare(Q, K, V, Q_lengths, K_lengths, W1, b1, W2, b2,
             ln1_g, ln1_b, ln2_g, ln2_b):
    Q = np.asarray(Q, dtype=np.float32)
    K = np.asarray(K, dtype=np.float32)
    V = np.asarray(V, dtype=np.float32)
    W1 = np.asarray(W1, dtype=np.float32)
    W2 = np.asarray(W2, dtype=np.float32)
    qlen = np.asarray(Q_lengths).astype(np.int64)
    klen = np.asarray(K_lengths).astype(np.int64)
    g1 = np.asarray(ln1_g, dtype=np.float32)
    b1v = np.asarray(ln1_b, dtype=np.float32)
    g2 = np.asarray(ln2_g, dtype=np.float32)
    b2v = np.asarray(ln2_b, dtype=np.float32)
    b1f = float(np.asarray(b1, dtype=np.float32).reshape(-1)[0])
    # b2 cancels exactly inside LN2.

    apply1 = not (np.all(g1 == 1.0) and np.all(b1v == 0.0))
    apply2 = not (np.all(g2 == 1.0) and np.all(b2v == 0.0))

    def tile_rows(x):
        # [B, R, C] -> [B, P, (R/P)*C] in the "(t p)" SBUF tile layout
        Bn, R, C = x.shape
        return np.ascontiguousarray(
            x.reshape(Bn, R // P, P, C).transpose(0, 2, 1, 3).reshape(
                Bn, P, (R // P) * C))

    qmask = (np.arange(QTL)[None, :] < qlen[:, None])
    QT = np.ascontiguousarray(Q.transpose(0, 2, 1)).astype(NP_BF16)
    QT = QT * qmask[:, None, :].astype(NP_BF16)
    QT = tile_rows(QT)
    KT = tile_rows(np.ascontiguousarray(K.transpose(0, 2, 1)).astype(NP_BF16))
    HQK = QT.shape[2] // 2
    QTA = np.ascontiguousarray(QT[:, :, :HQK])
    QTB = np.ascontiguousarray(QT[:, :, HQK:])
    KTA = np.ascontiguousarray(KT[:, :, :HQK])
    KTB = np.ascontiguousarray(KT[:, :, HQK:])
    # DoubleRow rhs wants the k-pair adjacent in memory: [B,P,kt2,d,i]
    Vt = V.astype(NP_F8).reshape(B, NKT // 2, 2, P, D)      # [B,kt2,i,p,d]
    V8 = np.ascontiguousarray(
        Vt.transpose(0, 3, 1, 4, 2)).reshape(B, P, NKT * D)  # p,kt2,d,i
    Qb = tile_rows(Q.astype(NP_BF16))
    qmb = qmask.astype(NP_BF16)
    kbb = np.where(np.arange(KTL)[None, :] < klen[:, None], 0.0, NEG
                   ).astype(NP_BF16)
    W1T = tile_rows(np.ascontiguousarray(W1.T).astype(NP_BF16)[None])[0]
    W2T = tile_rows(np.ascontiguousarray(W2.T).astype(NP_BF16)[None])[0]
    # JIT chunk layouts: W1C[ot] = [P, dt, 128], W2C[ch] = [P, ot, 512]
    W1C = np.ascontiguousarray(
        W1T.reshape(P, NDT, NDT, P).transpose(2, 0, 1, 3)).reshape(
        NDT, P, NDT * P)
    W2C = np.ascontiguousarray(
        W2T.reshape(P, NDT, NCH, 512).transpose(2, 0, 1, 3)).reshape(
        NCH, P, NDT * 512)

    nc = _build(apply1, apply2, b1f)

    in_maps = []
    for c in range(NCORES):
        s = slice(c * BL, (c + 1) * BL)
        m = {
            "QTAp": QTA[s], "QTBp": QTB[s], "KTAp": KTA[s],
            "KTBp": KTB[s], "V8p": V8[s], "Qp": Qb[s],
            "QMp": qmb[s], "KBp": kbb[s],
            "W1Cp": W1C, "W2Cp": W2C,
        }
        if apply1:
            m["G1p"] = g1
            m["B1p"] = b1v
        if apply2:
            m["G2p"] = g2
            m["B2p"] = b2v
        in_maps.append(m)

    return nc, in_maps


def kernel(**inputs):
    nc, in_maps = _prepare(**inputs)
    res = run_bass_kernel_spmd(nc, in_maps, list(range(NCORES)))
    out = np.concatenate(
        [res.results[c]["OUTp"].reshape(BL, QTL, D) for c in range(NCORES)],
        axis=0)
    return out.astype(np.float32)


# revision 31
# speedup vs baseline: 1.2162x; 1.0173x over previous
"""Trainium2 Bass kernel: single-head attention + FFN transformer block.

Matmuls in bf16 except U (fp8e4 DoubleRow); S^T layout (math per batch b):
  S^T[k,q] = sum_d K[d,k] Q'[d,q] + kb[k]*qm[q]   (Q' = Q^T, invalid-q cols
                                                   zeroed host-side; kb = 0
                                                   valid / -1e30 masked)
  E[k,q]   = exp(S^T/32 - ln16)     ACT, fp8e4 out. Valid rows: softmax numer
                                    scaled by 1/16 (cancels in the ratio);
                                    invalid q: E=1/16 uniform over ALL k ->
                                    att = mean(V), matching the reference.
  rowsum   = ones^T E (PE),  recip via DVE, scattered to [q-part] layout.
  U[q,d]   = E^T_tile V      fp8 DoubleRow, K=256/MM. V is pair-interleaved
                             host-side (k-pair adjacent bytes) so the moving
                             operand streams N columns, not 2N.
  qres     = att + Q         fused on DVE (scalar_tensor_tensor), bf16
  y        = LN1(qres)       per-qt chain: bn_stats/aggr on DVE, then
                             rsqrt = exp(-0.5*ln(var+eps)) on ACT (the
                             natural_log_exp table holds both; the table-set
                             picker is pinned below so the whole kernel does
                             ONE ACT_TABLE_LOAD), nm = -mu*r as one DVE STT
  yT       = XBAR dma transpose per qt, ALL on the sync queue (concurrent
             transposes on two HWDGE queues race on the crossbar and corrupt
             blocks; the crossbar is also slow ~85GB/s, so only y uses it)
  H^T[o,q] = relu(W1T yT + b1)   bf16, W1 streamed as 8 JIT column chunks
  Z[q,d]   = H^T W2T;   out = LN2(y + Z)  (b2 cancels inside LN2)

Sharding: data-parallel, 4 batches per core on 8 cores.

Emission order (= scheduler priority): attn(b+1) is hoisted ahead of
ffn1(b) for b>=1 so its S/U matmuls fill the PE while ffn1(b) waits on the
y(b)->yT chain; attn(1) stays after ffn1(0) because batch 1's inputs are
still on the wire at that point. The last batch runs FFN2's LN2 per-qt to
shorten the drain tail.

Queues (engine FIFO head-of-line blocking is the enemy):
  sync   = all XBAR transposes + output stores + QT/KT bulk + W chunks
           (no compute engine -> nothing latency-critical stalls)
  scalar = ACT compute ONLY, zero DMA dispatches (a paced dispatch on a
           backed-up ring blocks every ACT op behind it in the FIFO)
  gpsimd = tiny mask loads + QTB/V8/Qf bulk (SWDGE, ~105GB/s)

PSUM: 4 S banks (reused by FFN2's Zps) + 2 U + 2 FFN1 = 8.
"""

import sys

sys.path.insert(0, "/opt/trn_rl_repo")

import numpy as np
import ml_dtypes

import concourse.bass as bass
import concourse.bacc as bacc
import concourse.mybir as mybir
from concourse import tile
from concourse.tile import add_dep_helper
from concourse.bass_utils import run_bass_kernel_spmd

# Force every activation this kernel uses (Exp, Ln, Relu, Identity) to
# resolve to the one table set that contains them all; the default set
# picker bounces between exp-only and ln-only sets, costing a ~1.3us
# ACT_TABLE_LOAD per LayerNorm rsqrt (39 loads = 50us of ACT time).
import concourse.hw_specs as _hw_specs

_ORIG_GAT = _hw_specs.get_activation_tables
_SHARED_SET = "natural_log_exp_and_others"


def _gat_pinned(module_arch):
    tabs = _ORIG_GAT(module_arch)
    af = mybir.ActivationFunctionType
    ours = {af.Exp, af.Ln, af.Relu, af.Identity, af.Copy}
    return {
        name: (funcs if name == _SHARED_SET else funcs - ours)
        for name, funcs in tabs.items()
    }


_hw_specs.get_activation_tables = _gat_pinned
bacc.get_activation_tables = _gat_pinned

B, QTL, KTL, D = 32, 512, 512, 1024
NCORES = 8
BL = B // NCORES
P = 128
NQT = QTL // P   # 4
NKT = KTL // P   # 4
NDT = D // P     # 8
NCH = 2          # 512-wide psum chunks per 1024
EPS = 1e-5
NEG = -1.0e30
LN16 = 2.772588722239781

F32 = mybir.dt.float32
F16 = mybir.dt.float16
BF16 = mybir.dt.bfloat16
F8 = mybir.dt.float8e4
AF = mybir.ActivationFunctionType
ALU = mybir.AluOpType
DR = mybir.MatmulPerfMode.DoubleRow
NP_BF16 = ml_dtypes.bfloat16
NP_F8 = ml_dtypes.float8_e4m3


def _build(apply1: bool, apply2: bool, b1f: float):
    nc = bacc.Bacc(None, target_bir_lowering=False)

    HQK = (NDT // 2) * KTL
    QTAd = nc.dram_tensor("QTAp", [BL, P, HQK], BF16, kind="ExternalInput")
    QTBd = nc.dram_tensor("QTBp", [BL, P, HQK], BF16, kind="ExternalInput")
    KTAd = nc.dram_tensor("KTAp", [BL, P, HQK], BF16, kind="ExternalInput")
    KTBd = nc.dram_tensor("KTBp", [BL, P, HQK], BF16, kind="ExternalInput")
    Vd = nc.dram_tensor("V8p", [BL, P, NKT * D], F8, kind="ExternalInput")
    Qd = nc.dram_tensor("Qp", [BL, P, NQT * D], BF16, kind="ExternalInput")
    QMd = nc.dram_tensor("QMp", [BL, QTL], BF16, kind="ExternalInput")
    KBd = nc.dram_tensor("KBp", [BL, KTL], BF16, kind="ExternalInput")
    W1Cd = nc.dram_tensor("W1Cp", [NDT, P, NDT * P], BF16,
                          kind="ExternalInput")
    W2Cd = nc.dram_tensor("W2Cp", [NCH, P, NDT * 512], BF16,
                          kind="ExternalInput")
    if apply1:
        G1d = nc.dram_tensor("G1p", [D], F32, kind="ExternalInput")
        B1d = nc.dram_tensor("B1p", [D], F32, kind="ExternalInput")
    if apply2:
        G2d = nc.dram_tensor("G2p", [D], F32, kind="ExternalInput")
        B2d = nc.dram_tensor("B2p", [D], F32, kind="ExternalInput")
    OUTd = nc.dram_tensor("OUTp", [BL, NQT, P, D], F16, kind="ExternalOutput")

    with tile.TileContext(nc) as tc:
        with (
            tc.tile_pool(name="const", bufs=1) as pc,
            tc.tile_pool(name="wts", bufs=1) as pw,
            tc.tile_pool(name="qkin", bufs=2) as pin,
            tc.tile_pool(name="mid", bufs=3) as pmid,
            tc.tile_pool(name="eh", bufs=1) as peh,
            tc.tile_pool(name="stream", bufs=2) as pst,
            tc.tile_pool(name="small", bufs=2) as psm,
            tc.tile_pool(name="psS", bufs=1, space="PSUM") as psS,
            tc.tile_pool(name="psU", bufs=2, space="PSUM") as psU,
            tc.tile_pool(name="psF", bufs=2, space="PSUM") as psF,
        ):
            # ---- constants / warmup ----
            wz = pc.tile([P, P], BF16)
            nc.vector.memset(wz, 0.0)
            wz512 = pc.tile([P, 512], BF16)
            nc.vector.memset(wz512, 0.0)
            onesb = pc.tile([P, 1], F8)
            nc.vector.memset(onesb, 1.0)
            nln16b = pc.tile([P, 1], F32)
            nc.vector.memset(nln16b, -LN16)
            s32b = pc.tile([P, 1], F32)
            nc.vector.memset(s32b, 1.0 / 32.0)
            b1b = pc.tile([P, 1], F32)
            nc.vector.memset(b1b, b1f)
            epsb = pc.tile([P, 1], F32)
            nc.vector.memset(epsb, EPS)
            nhalfb = pc.tile([P, 1], F32)
            nc.vector.memset(nhalfb, -0.5)
            if apply1:
                g1t = pc.tile([P, D], F32)
                nc.gpsimd.dma_start(
                    g1t, bass.AP(tensor=G1d, offset=0, ap=[[0, P], [1, D]]))
                b1t = pc.tile([P, D], F32)
                nc.gpsimd.dma_start(
                    b1t, bass.AP(tensor=B1d, offset=0, ap=[[0, P], [1, D]]))
            if apply2:
                g2t = pc.tile([P, D], F32)
                nc.gpsimd.dma_start(
                    g2t, bass.AP(tensor=G2d, offset=0, ap=[[0, P], [1, D]]))
                b2t = pc.tile([P, D], F32)
                nc.gpsimd.dma_start(
                    b2t, bass.AP(tensor=B2d, offset=0, ap=[[0, P], [1, D]]))

            W1Ct = [pw.tile([P, NDT, P], BF16, name=f"w1c{ot}")
                    for ot in range(NDT)]
            W2Ct = [pw.tile([P, NDT, 512], BF16, name=f"w2c{ch}")
                    for ch in range(NCH)]

            def load_batch(b):
                # tiny mask loads first (they gate the S-bias matmuls),
                # then bulk; sync stays free for transposes + stores.
                qmt = psm.tile([1, QTL], BF16, tag="qm", bufs=3)
                nc.gpsimd.dma_start(qmt, QMd[b:b + 1, :])
                kbt = psm.tile([1, KTL], BF16, tag="kb", bufs=3)
                nc.gpsimd.dma_start(kbt, KBd[b:b + 1, :])
                QTsA = pin.tile([P, NDT // 2, QTL], BF16, tag="qta",
                                name=f"qtsa{b}")
                nc.sync.dma_start(QTsA.rearrange("p t q -> p (t q)"),
                                  QTAd[b])
                KTsA = pin.tile([P, NDT // 2, KTL], BF16, tag="kta",
                                name=f"ktsa{b}")
                nc.sync.dma_start(KTsA.rearrange("p t k -> p (t k)"),
                                  KTAd[b])
                QTsB = pin.tile([P, NDT // 2, QTL], BF16, tag="qtb",
                                name=f"qtsb{b}")
                nc.gpsimd.dma_start(QTsB.rearrange("p t q -> p (t q)"),
                                    QTBd[b])
                KTsB = pin.tile([P, NDT // 2, KTL], BF16, tag="ktb",
                                name=f"ktsb{b}")
                nc.sync.dma_start(KTsB.rearrange("p t k -> p (t k)"),
                                  KTBd[b])
                # scalar queue carries ZERO dma dispatches: ACT compute can
                # never stall behind a backed-up DMA ring
                V8 = pin.tile([P, NKT // 2, D, 2], F8, tag="v",
                              name=f"v8{b}")
                nc.gpsimd.dma_start(
                    V8.rearrange("p t d i -> p (t d i)"), Vd[b])
                Qf = pin.tile([P, NQT, D], BF16, tag="q", name=f"qf{b}")
                nc.gpsimd.dma_start(Qf.rearrange("p t d -> p (t d)"), Qd[b])
                return dict(QTsA=QTsA, QTsB=QTsB, KTsA=KTsA, KTsB=KTsB,
                            Qf=Qf, V8=V8, qmt=qmt, kbt=kbt)

            ins = {0: load_batch(0)}

            # PE warmup while batch-0 inputs stream in: keeps HAM busy and
            # flips it to 8/8 right as the first S matmuls arrive
            wps = psF.tile([P, 512], F32, tag="f", name="warm")
            for _ in range(16):
                nc.tensor.matmul(wps, wz, wz512, start=True, stop=True)

            state = {}

            def ln_scalars(mv, tagp, nqt=NQT, iters=0):
                # r = rsqrt(var+eps) = exp(-0.5*ln(var+eps)) on ACT (the
                # natural_log_exp table holds both -> no table reloads);
                # nm = -mu*r as ONE gpsimd STT (idle engine, so the serial
                # LN chain is not diluted by DVE/ACT bulk work).
                lnv = psm.tile([P, nqt], F32, tag=f"{tagp}lv")
                nc.scalar.activation(lnv, mv[:, :, 1], AF.Ln,
                                     bias=epsb[:, :])
                r = psm.tile([P, nqt], F32, tag=f"{tagp}r")
                nc.scalar.activation(r, lnv, AF.Exp,
                                     scale=nhalfb[:, :])
                nm = psm.tile([P, nqt], F32, tag=f"{tagp}nm")
                nc.vector.scalar_tensor_tensor(
                    nm, mv[:, :, 0], -1.0, r, op0=ALU.mult, op1=ALU.mult)
                return r, nm

            def attn(b):
                t = ins[b]
                QTh = (t["QTsA"], t["QTsB"])
                KTh = (t["KTsA"], t["KTsB"])
                V8, Qf = t["V8"], t["Qf"]
                qmt, kbt = t["qmt"], t["kbt"]
                last = b == BL - 1

                # --- S^T = K^T-tiles . Q'T (+ kb x qm), 4 psum banks ---
                Sps = [psS.tile([P, QTL], F32, tag=f"s{kt}",
                                name=f"sps{kt}_{b}")
                       for kt in range(NKT)]
                for dt in range(NDT):
                    for kt in range(NKT):
                        nc.tensor.matmul(
                            Sps[kt],
                            KTh[dt // 4][:, dt % 4, kt * P:(kt + 1) * P],
                            QTh[dt // 4][:, dt % 4, :],
                            start=(dt == 0), stop=False)
                for kt in range(NKT):
                    nc.tensor.matmul(
                        Sps[kt], kbt[:, kt * P:(kt + 1) * P], qmt[:, :],
                        start=False, stop=True)

                # --- E = exp(S/32 - ln16), fp8e4 ---
                E = peh.tile([P, NKT, QTL], F8, tag="e", name=f"e{b}")
                for kt in range(NKT):
                    nc.scalar.activation(
                        E[:, kt, :], Sps[kt], AF.Exp,
                        bias=nln16b[:, :], scale=s32b[:, :])

                # --- rowsum^T per q-tile (E_tile^T @ ones), recip ---
                rsps = psU.tile([P, NQT], F32, tag="u", name=f"rsps{b}")
                for qt in range(NQT):
                    for kt in range(NKT):
                        nc.tensor.matmul(
                            rsps[:, qt:qt + 1],
                            E[:, kt, qt * P:(qt + 1) * P],
                            onesb[:, :],
                            start=(kt == 0), stop=(kt == NKT - 1))
                recT = psm.tile([P, NQT], F32, tag="recT")
                nc.vector.reciprocal(recT, rsps)

                # --- U = E^T V (fp8 DoubleRow) + drain, LN1 stats ---
                qres_l = []
                st1_l = []
                for qt in range(NQT):
                    qres = pst.tile([P, D], BF16, tag="qres", bufs=4,
                                    name=f"qres{qt}_{b}")
                    qres_l.append(qres)
                    st1 = psm.tile([P, NCH, 6], F32, tag=f"st1{qt % 2}")
                    st1_l.append(st1)
                    for ch in range(NCH):
                        Ups = psU.tile([P, 512], F32, tag="u")
                        for k2 in range(NKT // 2):
                            rhs = V8[:, k2, ch * 512:(ch + 1) * 512, :]
                            nc.tensor.matmul(
                                Ups,
                                E[:, 2 * k2:2 * k2 + 2,
                                  qt * P:(qt + 1) * P],
                                rhs.rearrange("p d i -> p i d"),
                                start=(k2 == 0), stop=(k2 == NKT // 2 - 1),
                                perf_mode=DR)
                        qch = qres[:, ch * 512:(ch + 1) * 512]
                        nc.vector.scalar_tensor_tensor(
                            qch, Ups, recT[:, qt:qt + 1],
                            Qf[:, qt, ch * 512:(ch + 1) * 512],
                            op0=ALU.mult, op1=ALU.add)
                        nc.vector.bn_stats(st1[:, ch, :], qch)

                y = pmid.tile([P, NQT, D], BF16, tag="y", name=f"y{b}")
                YT4 = pmid.tile([P, NQT, NDT, P], BF16, tag="yt4",
                                name=f"yt4{b}")

                # per-qt chain (aggr->ln->y->transpose): the first yT
                # blocks FFN1, so keep the per-qt critical path short
                for qt in range(NQT):
                    mq = psm.tile([P, 1, 2], F32, tag=f"mv1q{qt % 2}")
                    nc.vector.bn_aggr(mq[:, 0, :], st1_l[qt])
                    r1s, nm1s = ln_scalars(mq, f"l1q{qt % 2}", nqt=1)
                    nc.scalar.activation(
                        y[:, qt, :], qres_l[qt], AF.Identity,
                        bias=nm1s[:, 0:1], scale=r1s[:, 0:1])
                    if apply1:
                        yf = y[:, qt, :]
                        nc.vector.tensor_mul(yf, yf, g1t)
                        nc.vector.tensor_add(yf, yf, b1t)
                    nc.sync.dma_start_transpose(YT4[:, qt, :, :],
                                                y[:, qt, :])
                state[b] = dict(y=y, YT4=YT4)

            def ffn1(b):
                YT4 = state[b]["YT4"]
                # --- FFN1: H^T = relu(W1T . yT + b1) ---
                HT = peh.tile([P, NDT, QTL], BF16, tag="ht", name=f"ht{b}")
                for ot in range(NDT):
                    Hps = psF.tile([P, QTL], F32, tag="f")
                    for dt in range(NDT):
                        nc.tensor.matmul(
                            Hps,
                            W1Ct[ot][:, dt, :],
                            YT4[:, :, dt, :],
                            start=(dt == 0), stop=(dt == NDT - 1))
                    nc.scalar.activation(HT[:, ot, :], Hps, AF.Relu,
                                         bias=b1b[:, :])
                state[b]["HT"] = HT

            def ffn2(b):
                y, HT = state[b]["y"], state[b]["HT"]
                last = b == BL - 1

                def fin2(qt, r2s, nm2, j=0):
                    stg = pst.tile([P, D], F16, tag="stg", bufs=2,
                                   name=f"stg{qt % 2}_{b}")
                    nc.scalar.activation(
                        stg, r2_l[qt], AF.Identity,
                        bias=nm2[:, j:j + 1], scale=r2s[:, j:j + 1])
                    if apply2:
                        nc.vector.tensor_mul(stg, stg, g2t)
                        nc.vector.tensor_add(stg, stg, b2t)
                    if last:
                        nc.sync.dma_start(OUTd[b][qt][:, :D // 2],
                                          stg[:, :D // 2])
                        nc.scalar.dma_start(OUTd[b][qt][:, D // 2:],
                                            stg[:, D // 2:])
                    else:
                        nc.sync.dma_start(OUTd[b][qt], stg)

                if not last:
                    mv2 = psm.tile([P, NQT, 2], F32, tag="mv2")
                r2_l = []
                for qt in range(NQT):
                    r2 = pst.tile([P, D], F32, tag="r2", bufs=4,
                                  name=f"r2_{qt}_{b}")
                    r2_l.append(r2)
                    st2 = psm.tile([P, NCH, 6], F32, tag="st2")
                    for ch in range(NCH):
                        Zps = psS.tile([P, 512], F32,
                                       tag=f"s{(qt * NCH + ch) % 4}",
                                       name=f"zps{qt}{ch}_{b}")
                        for ot in range(NDT):
                            nc.tensor.matmul(
                                Zps,
                                HT[:, ot, qt * P:(qt + 1) * P],
                                W2Ct[ch][:, ot, :],
                                start=(ot == 0), stop=(ot == NDT - 1))
                        rch = r2[:, ch * 512:(ch + 1) * 512]
                        nc.vector.scalar_tensor_tensor(
                            rch, Zps, 1.0,
                            y[:, qt, ch * 512:(ch + 1) * 512],
                            op0=ALU.mult, op1=ALU.add)
                        nc.vector.bn_stats(st2[:, ch, :], rch)
                    if last:
                        mvq = psm.tile([P, 1, 2], F32, tag=f"mv2q{qt % 2}")
                        nc.vector.bn_aggr(mvq[:, 0, :], st2)
                        r2s, nm2 = ln_scalars(mvq, f"l2q{qt % 2}", nqt=1,
                                              iters=2)
                        fin2(qt, r2s, nm2)
                    else:
                        nc.vector.bn_aggr(mv2[:, qt, :], st2)

                if not last:
                    r2s, nm2 = ln_scalars(mv2, "l2", iters=2)
                    for qt in range(NQT):
                        fin2(qt, r2s, nm2, j=qt)

            assert BL == 4
            for ot in range(NDT):
                nc.sync.dma_start(
                    W1Ct[ot].rearrange("p t c -> p (t c)"), W1Cd[ot])
            ins[1] = load_batch(1)
            attn(0)
            for ch in range(NCH):
                nc.sync.dma_start(
                    W2Ct[ch].rearrange("p t c -> p (t c)"), W2Cd[ch])
            ins[2] = load_batch(2)
            ffn1(0)
            attn(1)
            attn(2)
            ffn2(0)
            ins[3] = load_batch(3)
            ffn1(1)
            ffn2(1)
            attn(3)
            ffn1(2)
            ffn2(2)
            ffn1(3)
            ffn2(3)

    nc.finalize()
    return nc


def _prepare(Q, K, V, Q_lengths, K_lengths, W1, b1, W2, b2,
             ln1_g, ln1_b, ln2_g, ln2_b):
    Q = np.asarray(Q, dtype=np.float32)
    K = np.asarray(K, dtype=np.float32)
    V = np.asarray(V, dtype=np.float32)
    W1 = np.asarray(W1, dtype=np.float32)
    W2 = np.asarray(W2, dtype=np.float32)
    qlen = np.asarray(Q_lengths).astype(np.int64)
    klen = np.asarray(K_lengths).astype(np.int64)
    g1 = np.asarray(ln1_g, dtype=np.float32)
    b1v = np.asarray(ln1_b, dtype=np.float32)
    g2 = np.asarray(ln2_g, dtype=np.float32)
    b2v = np.asarray(ln2_b, dtype=np.float32)
    b1f = float(np.asarray(b1, dtype=np.float32).reshape(-1)[0])
    # b2 cancels exactly inside LN2.

    apply1 = not (np.all(g1 == 1.0) and np.all(b1v == 0.0))
    apply2 = not (np.all(g2 == 1.0) and np.all(b2v == 0.0))

    def tile_rows(x):
        # [B, R, C] -> [B, P, (R/P)*C] in the "(t p)" SBUF tile layout
        Bn, R, C = x.shape
        return np.ascontiguousarray(
            x.reshape(Bn, R // P, P, C).transpose(0, 2, 1, 3).reshape(
                Bn, P, (R // P) * C))

    qmask = (np.arange(QTL)[None, :] < qlen[:, None])
    QT = np.ascontiguousarray(Q.transpose(0, 2, 1)).astype(NP_BF16)
    QT = QT * qmask[:, None, :].astype(NP_BF16)
    QT = tile_rows(QT)
    KT = tile_rows(np.ascontiguousarray(K.transpose(0, 2, 1)).astype(NP_BF16))
    HQK = QT.shape[2] // 2
    QTA = np.ascontiguousarray(QT[:, :, :HQK])
    QTB = np.ascontiguousarray(QT[:, :, HQK:])
    KTA = np.ascontiguousarray(KT[:, :, :HQK])
    KTB = np.ascontiguousarray(KT[:, :, HQK:])
    # DoubleRow rhs wants the k-pair adjacent in memory: [B,P,kt2,d,i]
    Vt = V.astype(NP_F8).reshape(B, NKT // 2, 2, P, D)      # [B,kt2,i,p,d]
    V8 = np.ascontiguousarray(
        Vt.transpose(0, 3, 1, 4, 2)).reshape(B, P, NKT * D)  # p,kt2,d,i
    Qb = tile_rows(Q.astype(NP_BF16))
    qmb = qmask.astype(NP_BF16)
    kbb = np.where(np.arange(KTL)[None, :] < klen[:, None], 0.0, NEG
                   ).astype(NP_BF16)
    W1T = tile_rows(np.ascontiguousarray(W1.T).astype(NP_BF16)[None])[0]
    W2T = tile_rows(np.ascontiguousarray(W2.T).astype(NP_BF16)[None])[0]
    # JIT chunk layouts: W1C[ot] = [P, dt, 128], W2C[ch] = [P, ot, 512]
    W1C = np.ascontiguousarray(
        W1T.reshape(P, NDT, NDT, P).transpose(2, 0, 1, 3)).reshape(
        NDT, P, NDT * P)
    W2C = np.ascontiguousarray(
        W2T.reshape(P, NDT, NCH, 512).transpose(2, 0, 1, 3)).reshape(
        NCH, P, NDT * 512)

    nc = _build(apply1, apply2, b1f)

    in_maps = []
    for c in range(NCORES):
        s = slice(c * BL, (c + 1) * BL)
        m = {
            "QTAp": QTA[s], "QTBp": QTB[s], "KTAp": KTA[s],
            "KTBp": KTB[s], "V8p": V8[s], "Qp": Qb[s],
            "QMp": qmb[s], "KBp": kbb[s],
            "W1Cp": W1C, "W2Cp": W2C,
        }
        if apply1:
            m["G1p"] = g1
            m["B1p"] = b1v
        if apply2:
            m["G2p"] = g2
            m["B2p"] = b2v
        in_maps.append(m)

    return nc, in_maps


def kernel(**inputs):
    nc, in_maps = _prepare(**inputs)
    res = run_bass_kernel_spmd(nc, in_maps, list(range(NCORES)))
    out = np.concatenate(
        [res.results[c]["OUTp"].reshape(BL, QTL, D) for c in range(NCORES)],
        axis=0)
    return out.astype(np.float32)
